# revision 1
# baseline (speedup 1.0000x reference)
"""BidafAttention Trainium2 kernel.

score[b,l,r] = tanh( (lhs*w_prod) @ rhs^T + (lhs@w_l)[:,None] + (rhs@w_r)[None,:] + b )
a_lhs = softmax_R(score); a_rhs = softmax_L(score)
lhs_out = concat([lhs, a_lhs @ rhs], -1); rhs_out = concat([rhs, a_rhs^T @ lhs], -1)

Strategy: data-parallel over batch N=16 -> 2 batches per NeuronCore.
Host-side sharding also lays the operands out for the device (bf16 cast,
w_prod fold, d-major copies for the score matmul, tiny u/v matvecs).
On device (per batch, all matmuls bf16 with fp32 PSUM accumulation):
  - S tiles = lhsT^T @ rhsT (+ v via identity-matmul; u folded into the
    tanh bias); scores are tanh-bounded so softmax needs no max pass
  - E = exp(tanh(S)) in bf16; rowsum via activation accum_out;
    E^T via PE transposes; colsum via accum_out on the E^T copies
  - att_lhs = diag(1/rowsum) @ (E @ rhs); att_rhs = diag(1/colsum) @ (E^T @ lhs)
"""

import sys

for _p in ("/opt/trn_rl_repo",):
    if _p not in sys.path:
        sys.path.insert(0, _p)

import numpy as np
import ml_dtypes

import concourse.tile as tile
import concourse.mybir as mybir
from concourse import bacc
from concourse.bass_utils import run_bass_kernel_spmd

AF = mybir.ActivationFunctionType
BF16 = mybir.dt.bfloat16
F32 = mybir.dt.float32

P = 128
SEQ = 1024  # L == R == D == 1024
NT = SEQ // P  # 8 tiles per dim
CH = 512  # psum chunk (free dim)
NCH = SEQ // CH  # 2
NB = 2  # batches per core
N_CORES = 8
D = 1024
N_WARMUP = 38  # dummy PE ops at start to lift the HAM clock gate

_nc_cache = None


def _build_program():
    nc = bacc.Bacc("TRN2", target_bir_lowering=False, debug=False, num_devices=N_CORES)

    # natural bf16 inputs (moving operands of the output matmuls) and
    # d-major bf16 copies (score matmul operands; lhsT carries w_prod)
    lhs_n = nc.declare_dram_parameter("lhs_n", [NB, SEQ, D], BF16, isOutput=False)
    rhs_n = nc.declare_dram_parameter("rhs_n", [NB, SEQ, D], BF16, isOutput=False)
    lhs_t = nc.declare_dram_parameter("lhs_t", [NB, D, SEQ], BF16, isOutput=False)
    rhs_t = nc.declare_dram_parameter("rhs_t", [NB, D, SEQ], BF16, isOutput=False)
    u_d = nc.declare_dram_parameter("u", [NB, P, NT], F32, isOutput=False)
    vb_d = nc.declare_dram_parameter("vb", [NB, P, SEQ], BF16, isOutput=False)
    id_d = nc.declare_dram_parameter("id_bf", [P, P], BF16, isOutput=False)
    att_lhs = nc.declare_dram_parameter("att_lhs", [NB, SEQ, D], F32, isOutput=True)
    att_rhs = nc.declare_dram_parameter("att_rhs", [NB, SEQ, D], F32, isOutput=True)

    from contextlib import ExitStack

    with tile.TileContext(nc) as tc, ExitStack() as ctx:
        const = ctx.enter_context(tc.tile_pool(name="const", bufs=1))
        ident = const.tile([P, P], BF16)
        nc.sync.dma_start(ident[:], id_d[:])

        pool_in = ctx.enter_context(tc.tile_pool(name="inbf", bufs=2))
        pool_tr = ctx.enter_context(tc.tile_pool(name="trbf", bufs=1))
        pool_e = ctx.enter_context(tc.tile_pool(name="ebf", bufs=1))
        pool_T = ctx.enter_context(tc.tile_pool(name="tanh", bufs=9))
        pool_sm = ctx.enter_context(tc.tile_pool(name="small", bufs=2))
        pool_out = ctx.enter_context(tc.tile_pool(name="osb", bufs=6))
        pool_dram = ctx.enter_context(tc.tile_pool(name="scr", bufs=1, space="DRAM"))
        psum_s = ctx.enter_context(tc.tile_pool(name="ps_s", bufs=2, space="PSUM"))
        psum_tr = ctx.enter_context(tc.tile_pool(name="ps_tr", bufs=2, space="PSUM"))
        psum_o = ctx.enter_context(tc.tile_pool(name="ps_o", bufs=4, space="PSUM"))

        # --- PE warmup: keep TensorE busy from t=0 so the HAM clock gate
        # opens (1.2 -> 2.4 GHz) before the first real matmul arrives.
        wps = psum_tr.tile([P, CH], BF16, tag="ptr", name="warm_ps")
        for _ in range(N_WARMUP):
            nc.tensor.transpose(wps[:, 0:P], ident[:], ident[:])
        wsb = const.tile([P, P], BF16, name="warm_sb")
        nc.scalar.copy(wsb[:], wps[:, 0:P])

        for b in range(NB):
            lhs_bf = pool_in.tile([P, NT, SEQ], BF16, tag="lhs_bf", name=f"lhs_bf{b}")
            rhs_bf = pool_in.tile([P, NT, SEQ], BF16, tag="rhs_bf", name=f"rhs_bf{b}")
            lhsT = pool_tr.tile([P, NT, SEQ], BF16, tag="lhsT", name=f"lhsT{b}")
            rhsT = pool_tr.tile([P, NT, SEQ], BF16, tag="rhsT", name=f"rhsT{b}")
            # transposed operands first (the score matmuls need them), in
            # column-halves ordered so the jc=0 score chunk unblocks after
            # the first 2 MB of loads
            u_sb = pool_sm.tile([P, NT], F32, tag="u", name=f"u{b}")
            nc.sync.dma_start(u_sb[:], u_d[b])
            vb_sb = pool_sm.tile([P, SEQ], BF16, tag="vb", name=f"vb{b}")
            nc.sync.dma_start(vb_sb[:], vb_d[b])
            for half in range(2):
                sl = slice(half * CH, (half + 1) * CH)
                for k in range(NT):
                    nc.sync.dma_start(lhsT[:, k, sl], lhs_t[b, k * P:(k + 1) * P, sl])
                for k in range(NT):
                    nc.sync.dma_start(rhsT[:, k, sl], rhs_t[b, k * P:(k + 1) * P, sl])
            for i in range(NT):
                nc.sync.dma_start(lhs_bf[:, i, :], lhs_n[b, i * P:(i + 1) * P, :])
                nc.sync.dma_start(rhs_bf[:, i, :], rhs_n[b, i * P:(i + 1) * P, :])

            E = pool_e.tile([P, NT, SEQ], BF16, tag="E", name=f"E{b}")
            E_T = pool_e.tile([P, NT, SEQ], BF16, tag="E_T", name=f"E_T{b}")
            rowsum = pool_sm.tile([P, NT], F32, tag="rowsum", name=f"rowsum{b}")
            cparts = pool_sm.tile([P, 2, NT], F32, tag="cparts", name=f"cparts{b}")

            T_ts = [
                pool_T.tile([P, SEQ], F32, tag="T", name=f"T{b}_{i}")
                for i in range(NT)
            ]
            for jc in range(NCH):
                for i in range(NT):
                    S_ps = psum_s.tile([P, CH], F32, tag="ps", name=f"S{b}_{i}_{jc}")
                    for k in range(NT):
                        nc.tensor.matmul(
                            S_ps[:],
                            lhsT[:, k, i * P:(i + 1) * P],
                            rhsT[:, k, jc * CH:(jc + 1) * CH],
                            start=(k == 0),
                            stop=(k == NT - 1),
                        )
                    # += v[r] on DVE (keeps TensorE for real matmuls)
                    nc.vector.tensor_add(
                        S_ps[:], S_ps[:], vb_sb[:, jc * CH:(jc + 1) * CH]
                    )
                    # T = tanh(S + u[l]); u enters as the per-partition bias
                    nc.scalar.activation(
                        T_ts[i][:, jc * CH:(jc + 1) * CH],
                        S_ps[:],
                        AF.Tanh,
                        bias=u_sb[:, i:i + 1],
                    )
                    if jc == NCH - 1:
                        # E = exp(T) (bf16) + rowsum for softmax_R in one pass
                        nc.scalar.activation(
                            E[:, i, :], T_ts[i][:], AF.Exp,
                            accum_out=rowsum[:, i:i + 1],
                        )

            r_row = pool_sm.tile([P, NT], F32, tag="rrow", name=f"rrow{b}")
            nc.vector.reciprocal(r_row[:], rowsum[:])
            r_col = pool_sm.tile([P, NT], F32, tag="rcol", name=f"rcol{b}")

            # att_rhs[r, d] = (1/colsum[r]) * sum_l E[l,r] lhs[l,d]
            # 4-transpose bursts alternate with the matmul groups so the
            # transposes' LDWEIGHTS pull ahead under the N=512 streams
            # (PE's reorder window); colsum rides the E^T copies' accum_out.
            for j in range(NT):
                po_tiles = []
                for half in range(2):
                    pt = psum_tr.tile([P, CH], BF16, tag="ptr", name=f"pte{b}_{j}_{half}")
                    for q in range(4):
                        i = half * 4 + q
                        nc.tensor.transpose(
                            pt[:, q * P:(q + 1) * P],
                            E[:, i, j * P:(j + 1) * P],
                            ident[:],
                        )
                    # copy + partial colsum (sum over this 512-wide l-chunk)
                    nc.scalar.activation(
                        E_T[:, j, half * CH:(half + 1) * CH],
                        pt[:],
                        AF.Copy,
                        accum_out=cparts[:, half, j:j + 1],
                    )
                    dc = half
                    po = psum_o.tile([P, CH], F32, tag="po", name=f"por{b}_{j}_{dc}")
                    for k in range(NT):
                        nc.tensor.matmul(
                            po[:],
                            E[:, k, j * P:(j + 1) * P],
                            lhs_bf[:, k, dc * CH:(dc + 1) * CH],
                            start=(k == 0),
                            stop=(k == NT - 1),
                        )
                    po_tiles.append(po)
                nc.vector.tensor_add(
                    r_col[:, j:j + 1], cparts[:, 0, j:j + 1], cparts[:, 1, j:j + 1]
                )
                nc.vector.reciprocal(r_col[:, j:j + 1], r_col[:, j:j + 1])
                for dc in range(NCH):
                    osb = pool_out.tile([P, CH], F32, tag="osb", name=f"or{b}_{j}_{dc}")
                    nc.scalar.mul(osb[:], po_tiles[dc][:], r_col[:, j:j + 1])
                    nc.sync.dma_start(
                        att_rhs[b, j * P:(j + 1) * P, dc * CH:(dc + 1) * CH], osb[:]
                    )

            # att_lhs[l, d] = (1/rowsum[l]) * sum_r E[l,r] rhs[r,d]
            for i in range(NT):
                for dc in range(NCH):
                    po = psum_o.tile([P, CH], F32, tag="po", name=f"pol{b}_{i}_{dc}")
                    for k in range(NT):
                        nc.tensor.matmul(
                            po[:],
                            E_T[:, k, i * P:(i + 1) * P],
                            rhs_bf[:, k, dc * CH:(dc + 1) * CH],
                            start=(k == 0),
                            stop=(k == NT - 1),
                        )
                    osb = pool_out.tile([P, CH], F32, tag="osb", name=f"ol{b}_{i}_{dc}")
                    nc.vector.tensor_scalar_mul(osb[:], po[:], r_row[:, i:i + 1])
                    nc.sync.dma_start(
                        att_lhs[b, i * P:(i + 1) * P, dc * CH:(dc + 1) * CH], osb[:]
                    )

        # warmup sink: a DRAM write keeps the warmup chain live; emitted
        # last so no real DMA ever queues behind the warmup dependency
        warm_dram = pool_dram.tile([P, P], BF16, tag="warm", name="warm_dram")
        nc.sync.dma_start(warm_dram[:], wsb[:])

    nc.compile()
    return nc


def _get_nc():
    global _nc_cache
    if _nc_cache is None:
        _nc_cache = _build_program()
    return _nc_cache


def _prepare_in_maps(lhs, rhs, w, b):
    lhs = np.ascontiguousarray(lhs, dtype=np.float32)
    rhs = np.ascontiguousarray(rhs, dtype=np.float32)
    w = np.asarray(w, dtype=np.float32)
    b = np.float32(b)
    w_prod, w_l, w_r = w[:D], w[D:2 * D], w[2 * D:]

    # tiny host matvecs (exact, fp32)
    u_full = lhs @ w_l + b  # (N, L)
    v_full = rhs @ w_r      # (N, R)

    bf = ml_dtypes.bfloat16
    id_bf = np.eye(P, dtype=bf)
    lhs_n = lhs.astype(bf)
    rhs_n = rhs.astype(bf)
    # d-major score operands; w_prod folds into lhs^T
    lhs_t = np.ascontiguousarray(
        (lhs_n.astype(np.float32) * w_prod).transpose(0, 2, 1)
    ).astype(bf)
    rhs_t = np.ascontiguousarray(rhs_n.transpose(0, 2, 1))

    in_maps = []
    for c in range(N_CORES):
        b0 = c * NB
        u_arr = np.ascontiguousarray(
            u_full[b0:b0 + NB].reshape(NB, NT, P).transpose(0, 2, 1)
        )  # (NB, 128, 8)
        v_bf = v_full[b0:b0 + NB].astype(bf)  # (NB, R)
        vb_arr = np.ascontiguousarray(
            np.broadcast_to(v_bf[:, None, :], (NB, P, SEQ))
        )
        in_maps.append(
            {
                "lhs_n": lhs_n[b0:b0 + NB],
                "rhs_n": rhs_n[b0:b0 + NB],
                "lhs_t": lhs_t[b0:b0 + NB],
                "rhs_t": rhs_t[b0:b0 + NB],
                "u": u_arr,
                "vb": vb_arr,
                "id_bf": id_bf,
            }
        )
    return in_maps


def run_device(lhs, rhs, w, b, trace=False):
    """Returns (att_lhs, att_rhs, BassKernelResults)."""
    nc = _get_nc()
    in_maps = _prepare_in_maps(lhs, rhs, w, b)
    res = run_bass_kernel_spmd(
        nc, in_maps, core_ids=list(range(N_CORES)), trace=trace
    )
    N = lhs.shape[0]
    att_lhs = np.empty((N, SEQ, D), dtype=np.float32)
    att_rhs = np.empty((N, SEQ, D), dtype=np.float32)
    for c in range(N_CORES):
        b0 = c * NB
        att_lhs[b0:b0 + NB] = res.results[c]["att_lhs"]
        att_rhs[b0:b0 + NB] = res.results[c]["att_rhs"]
    return att_lhs, att_rhs, res


def kernel(lhs, rhs, w, b):
    import os

    lhs = np.asarray(lhs, dtype=np.float32)
    rhs = np.asarray(rhs, dtype=np.float32)
    assert lhs.shape == (N_CORES * NB, SEQ, D) and rhs.shape == lhs.shape, (
        f"expected ({N_CORES * NB}, {SEQ}, {D}) inputs, got {lhs.shape}/{rhs.shape}"
    )
    had = os.environ.get("BASS_NEVER_TRACE")
    os.environ["BASS_NEVER_TRACE"] = "1"
    try:
        att_lhs, att_rhs, _ = run_device(lhs, rhs, w, b, trace=False)
    finally:
        if had is None:
            os.environ.pop("BASS_NEVER_TRACE", None)
        else:
            os.environ["BASS_NEVER_TRACE"] = had
    lhs_out = np.concatenate([lhs, att_lhs], axis=2)
    rhs_out = np.concatenate([rhs, att_rhs], axis=2)
    return lhs_out, rhs_out



# revision 4
# speedup vs baseline: 1.4243x; 1.4243x over previous
"""BidafAttention Trainium2 kernel (fp8 DoubleRow version).

score[b,l,r] = tanh( (lhs*w_prod) @ rhs^T + (lhs@w_l)[:,None] + (rhs@w_r)[None,:] + b )
a_lhs = softmax_R(score); a_rhs = softmax_L(score)
lhs_out = concat([lhs, a_lhs @ rhs], -1); rhs_out = concat([rhs, a_rhs^T @ lhs], -1)

Strategy: data-parallel over batch N=16 -> 2 batches per NeuronCore.
All heavy matmuls run in fp8e4 with perf_mode=DoubleRow (two K=128
contraction tiles per instruction -> ~1.8x PE throughput vs bf16).
Quantization scheme: w_prod is sqrt-split over both score operands
(lhs' = lhs*sign(w)*sqrt|w|*8, rhs' = rhs*sqrt|w|*8) so both stay in
e4m3's normal range; the resulting 64x score scale is removed by the
tanh activation's free scale factor. v (per-r bias) enters the score
PSUM via a K=1 ones-matmul; u (per-l) rides the tanh bias input.
E = exp(tanh(.)) is written as fp8 directly with rowsum via accum_out;
E^T via PE transposes (fp8), colsum rides the E^T copies' accum_out.
Outputs are computed in fp32 PSUM, scaled by 1/rowsum (1/colsum) on
DVE during PSUM->SBUF evacuation, and DMA'd out as bf16.
"""

import sys

for _p in ("/opt/trn_rl_repo",):
    if _p not in sys.path:
        sys.path.insert(0, _p)

import numpy as np
import ml_dtypes

import concourse.tile as tile
import concourse.mybir as mybir
from concourse import bacc
from concourse.bass_utils import run_bass_kernel_spmd

AF = mybir.ActivationFunctionType
DR = mybir.MatmulPerfMode.DoubleRow
BF16 = mybir.dt.bfloat16
F32 = mybir.dt.float32
F8 = mybir.dt.float8e4

P = 128
SEQ = 1024  # L == R == D == 1024
NT = SEQ // P  # 8 tiles per dim
NKP = NT // 2  # 4 DoubleRow k-pairs
CH = 512  # psum chunk (free dim)
NCH = SEQ // CH  # 2
NB = 2  # batches per core
N_CORES = 8
D = 1024
SCL = 64.0  # score scale from the 8x-per-side fp8 quantization scaling
N_WARMUP = 38  # dummy PE ops at start to lift the HAM clock gate

_nc_cache = None


def _build_program():
    nc = bacc.Bacc("TRN2", target_bir_lowering=False, debug=False, num_devices=N_CORES)

    lhs_n = nc.declare_dram_parameter("lhs_n", [NB, SEQ, D], F8, isOutput=False)
    rhs_n = nc.declare_dram_parameter("rhs_n", [NB, SEQ, D], F8, isOutput=False)
    lhs_t = nc.declare_dram_parameter("lhs_t", [NB, D, SEQ], F8, isOutput=False)
    rhs_t = nc.declare_dram_parameter("rhs_t", [NB, D, SEQ], F8, isOutput=False)
    u_d = nc.declare_dram_parameter("u", [NB, P, NT], F32, isOutput=False)
    v_d = nc.declare_dram_parameter("v64", [NB, 1, SEQ], BF16, isOutput=False)
    id_d = nc.declare_dram_parameter("id8", [P, P], F8, isOutput=False)
    att_lhs = nc.declare_dram_parameter("att_lhs", [NB, SEQ, D], BF16, isOutput=True)
    att_rhs = nc.declare_dram_parameter("att_rhs", [NB, SEQ, D], BF16, isOutput=True)

    from contextlib import ExitStack

    with tile.TileContext(nc) as tc, ExitStack() as ctx:
        const = ctx.enter_context(tc.tile_pool(name="const", bufs=1))
        ident = const.tile([P, P], F8)
        nc.sync.dma_start(ident[:], id_d[:])
        ones1 = const.tile([1, P], BF16, name="ones1")
        nc.any.memset(ones1[:], 1.0)

        pool_in = ctx.enter_context(tc.tile_pool(name="inf8", bufs=2))
        pool_tr = ctx.enter_context(tc.tile_pool(name="trf8", bufs=2))
        pool_e = ctx.enter_context(tc.tile_pool(name="ef8", bufs=2))
        pool_T = ctx.enter_context(tc.tile_pool(name="tanh", bufs=4))
        pool_sm = ctx.enter_context(tc.tile_pool(name="small", bufs=2))
        pool_out = ctx.enter_context(tc.tile_pool(name="osb", bufs=4))
        pool_dram = ctx.enter_context(tc.tile_pool(name="scr", bufs=1, space="DRAM"))
        psum_s = ctx.enter_context(tc.tile_pool(name="ps_s", bufs=2, space="PSUM"))
        psum_tr = ctx.enter_context(tc.tile_pool(name="ps_tr", bufs=2, space="PSUM"))
        psum_o = ctx.enter_context(tc.tile_pool(name="ps_o", bufs=4, space="PSUM"))

        # --- PE warmup: keep TensorE busy from t=0 so the HAM clock gate
        # opens (1.2 -> 2.4 GHz) before the first real matmul arrives.
        wps = psum_tr.tile([P, CH, 2], F8, tag="ptr", name="warm_ps")
        for _ in range(N_WARMUP):
            nc.tensor.transpose(wps[:, 0:P, 0], ident[:], ident[:])
        wsb = const.tile([P, P], F8, name="warm_sb")
        nc.scalar.copy(wsb[:], wps[:, 0:P, 0])

        for b in range(NB):
            lhs_nat = pool_in.tile([P, NT, SEQ], F8, tag="lhs_nat", name=f"lhs_nat{b}")
            rhs_nat = pool_in.tile([P, NT, SEQ], F8, tag="rhs_nat", name=f"rhs_nat{b}")
            lhsT = pool_tr.tile([P, NT, SEQ], F8, tag="lhsT", name=f"lhsT{b}")
            rhsT = pool_tr.tile([P, NT, SEQ], F8, tag="rhsT", name=f"rhsT{b}")
            u_sb = pool_sm.tile([P, NT], F32, tag="u", name=f"u{b}")
            nc.sync.dma_start(u_sb[:], u_d[b])
            v_sb = pool_sm.tile([1, SEQ], BF16, tag="v", name=f"v{b}")
            nc.sync.dma_start(v_sb[:], v_d[b])
            # transposed (d-major) operands first: the score matmuls need
            # them. One descriptor per column-half, l/r half 0 first so the
            # jc=0 score chunks unblock as early as possible.
            for half in range(2):
                sl = slice(half * CH, (half + 1) * CH)
                nc.sync.dma_start(
                    lhsT[:, :, sl],
                    lhs_t[b].rearrange("(k p) x -> p k x", p=P)[:, :, sl],
                )
                nc.sync.dma_start(
                    rhsT[:, :, sl],
                    rhs_t[b].rearrange("(k p) x -> p k x", p=P)[:, :, sl],
                )
            nc.sync.dma_start(
                lhs_nat[:, :, :], lhs_n[b].rearrange("(k p) x -> p k x", p=P)
            )
            nc.sync.dma_start(
                rhs_nat[:, :, :], rhs_n[b].rearrange("(k p) x -> p k x", p=P)
            )

            E = pool_e.tile([P, NT, SEQ], F8, tag="E", name=f"E{b}")
            E_T = pool_e.tile([P, NT, SEQ], F8, tag="E_T", name=f"E_T{b}")
            rparts = pool_sm.tile([P, 2, NT], F32, tag="rparts", name=f"rparts{b}")
            cparts = pool_sm.tile([P, 2, NT], F32, tag="cparts", name=f"cparts{b}")

            # --- score: S = (lhs')^T @ rhs' (+ v via K=1 ones-matmul),
            # T = tanh(S/64 + u), E = exp(T) with rowsum partials
            for jc in range(NCH):
                for i in range(NT):
                    S_ps = psum_s.tile([P, CH], F32, tag="ps", name=f"S{b}_{i}_{jc}")
                    for kp in range(NKP):
                        nc.tensor.matmul(
                            S_ps[:],
                            lhsT[:, 2 * kp:2 * kp + 2, i * P:(i + 1) * P],
                            rhsT[:, 2 * kp:2 * kp + 2, jc * CH:(jc + 1) * CH],
                            start=(kp == 0),
                            stop=False,
                            perf_mode=DR,
                        )
                    nc.tensor.matmul(
                        S_ps[:],
                        ones1[0:1, :],
                        v_sb[0:1, jc * CH:(jc + 1) * CH],
                        start=False,
                        stop=True,
                    )
                    T_scr = pool_T.tile([P, CH], F32, tag="T", name=f"T{b}_{i}_{jc}")
                    nc.scalar.activation(
                        T_scr[:],
                        S_ps[:],
                        AF.Tanh,
                        bias=u_sb[:, i:i + 1],
                        scale=1.0 / SCL,
                    )
                    nc.scalar.activation(
                        E[:, i, jc * CH:(jc + 1) * CH],
                        T_scr[:],
                        AF.Exp,
                        accum_out=rparts[:, jc, i:i + 1],
                    )

            rowsum = pool_sm.tile([P, NT], F32, tag="rowsum", name=f"rowsum{b}")
            nc.vector.tensor_add(rowsum[:], rparts[:, 0, :], rparts[:, 1, :])
            r_row = pool_sm.tile([P, NT], F32, tag="rrow", name=f"rrow{b}")
            nc.vector.reciprocal(r_row[:], rowsum[:])
            r_col = pool_sm.tile([P, NT], F32, tag="rcol", name=f"rcol{b}")

            # att_rhs[r, d] = (1/colsum[r]) * sum_l E[l,r] lhs[l,d]
            # 4-transpose bursts alternate with the matmul groups; colsum
            # rides the E^T copies' accum_out.
            for j in range(NT):
                po_tiles = []
                for half in range(2):
                    # fp8 PE transpose requires output element step 2: write
                    # plane 0 of a [P, CH, 2] psum tile, read it back strided.
                    pt = psum_tr.tile([P, CH, 2], F8, tag="ptr", name=f"pte{b}_{j}_{half}")
                    for q in range(4):
                        i = half * 4 + q
                        nc.tensor.transpose(
                            pt[:, q * P:(q + 1) * P, 0],
                            E[:, i, j * P:(j + 1) * P],
                            ident[:],
                        )
                    nc.scalar.activation(
                        E_T[:, j, half * CH:(half + 1) * CH],
                        pt[:, :, 0],
                        AF.Copy,
                        accum_out=cparts[:, half, j:j + 1],
                    )
                    dc = half
                    po = psum_o.tile([P, CH], F32, tag="po", name=f"por{b}_{j}_{dc}")
                    for kp in range(NKP):
                        nc.tensor.matmul(
                            po[:],
                            E[:, 2 * kp:2 * kp + 2, j * P:(j + 1) * P],
                            lhs_nat[:, 2 * kp:2 * kp + 2, dc * CH:(dc + 1) * CH],
                            start=(kp == 0),
                            stop=(kp == NKP - 1),
                            perf_mode=DR,
                        )
                    po_tiles.append(po)
                nc.vector.tensor_add(
                    r_col[:, j:j + 1], cparts[:, 0, j:j + 1], cparts[:, 1, j:j + 1]
                )
                nc.vector.reciprocal(r_col[:, j:j + 1], r_col[:, j:j + 1])
                osb = pool_out.tile([P, SEQ], BF16, tag="osb", name=f"or{b}_{j}")
                for dc in range(NCH):
                    nc.vector.tensor_scalar_mul(
                        osb[:, dc * CH:(dc + 1) * CH], po_tiles[dc][:], r_col[:, j:j + 1]
                    )
                nc.sync.dma_start(att_rhs[b, j * P:(j + 1) * P, :], osb[:])

            # att_lhs[l, d] = (1/rowsum[l]) * sum_r E[l,r] rhs[r,d]
            for i in range(NT):
                osb = pool_out.tile([P, SEQ], BF16, tag="osb", name=f"ol{b}_{i}")
                for dc in range(NCH):
                    po = psum_o.tile([P, CH], F32, tag="po", name=f"pol{b}_{i}_{dc}")
                    for kp in range(NKP):
                        nc.tensor.matmul(
                            po[:],
                            E_T[:, 2 * kp:2 * kp + 2, i * P:(i + 1) * P],
                            rhs_nat[:, 2 * kp:2 * kp + 2, dc * CH:(dc + 1) * CH],
                            start=(kp == 0),
                            stop=(kp == NKP - 1),
                            perf_mode=DR,
                        )
                    nc.vector.tensor_scalar_mul(
                        osb[:, dc * CH:(dc + 1) * CH], po[:], r_row[:, i:i + 1]
                    )
                nc.sync.dma_start(att_lhs[b, i * P:(i + 1) * P, :], osb[:])

        # warmup sink: a DRAM write keeps the warmup chain live; emitted
        # last so no real DMA ever queues behind the warmup dependency
        warm_dram = pool_dram.tile([P, P], F8, tag="warm", name="warm_dram")
        nc.sync.dma_start(warm_dram[:], wsb[:])

    nc.compile()
    return nc


def _get_nc():
    global _nc_cache
    if _nc_cache is None:
        _nc_cache = _build_program()
    return _nc_cache


def _prepare_in_maps(lhs, rhs, w, b):
    lhs = np.ascontiguousarray(lhs, dtype=np.float32)
    rhs = np.ascontiguousarray(rhs, dtype=np.float32)
    w = np.asarray(w, dtype=np.float32)
    b = np.float32(b)
    w_prod, w_l, w_r = w[:D], w[D:2 * D], w[2 * D:]

    # tiny host matvecs (exact, fp32)
    u_full = lhs @ w_l + b  # (N, L)
    v_full = rhs @ w_r      # (N, R)

    f8 = ml_dtypes.float8_e4m3
    bf = ml_dtypes.bfloat16
    id8 = np.eye(P, dtype=f8)
    lhs_n8 = lhs.astype(f8)
    rhs_n8 = rhs.astype(f8)
    # d-major score operands; w_prod sqrt-split over both sides, 8x
    # per-side scale keeps e4m3 operands in the normal range.
    sq = np.sqrt(np.abs(w_prod))
    lhs_t8 = np.ascontiguousarray(
        (lhs * (np.sign(w_prod) * sq * 8.0)).transpose(0, 2, 1)
    ).astype(f8)
    rhs_t8 = np.ascontiguousarray((rhs * (sq * 8.0)).transpose(0, 2, 1)).astype(f8)

    in_maps = []
    for c in range(N_CORES):
        b0 = c * NB
        u_arr = np.ascontiguousarray(
            u_full[b0:b0 + NB].reshape(NB, NT, P).transpose(0, 2, 1)
        )  # (NB, 128, 8)
        v64 = np.ascontiguousarray(
            (v_full[b0:b0 + NB] * SCL).astype(bf).reshape(NB, 1, SEQ)
        )
        in_maps.append(
            {
                "lhs_n": lhs_n8[b0:b0 + NB],
                "rhs_n": rhs_n8[b0:b0 + NB],
                "lhs_t": lhs_t8[b0:b0 + NB],
                "rhs_t": rhs_t8[b0:b0 + NB],
                "u": u_arr,
                "v64": v64,
                "id8": id8,
            }
        )
    return in_maps


def run_device(lhs, rhs, w, b, trace=False):
    """Returns (att_lhs, att_rhs, BassKernelResults)."""
    nc = _get_nc()
    in_maps = _prepare_in_maps(lhs, rhs, w, b)
    res = run_bass_kernel_spmd(
        nc, in_maps, core_ids=list(range(N_CORES)), trace=trace
    )
    N = lhs.shape[0]
    att_lhs = np.empty((N, SEQ, D), dtype=np.float32)
    att_rhs = np.empty((N, SEQ, D), dtype=np.float32)
    for c in range(N_CORES):
        b0 = c * NB
        att_lhs[b0:b0 + NB] = res.results[c]["att_lhs"].astype(np.float32)
        att_rhs[b0:b0 + NB] = res.results[c]["att_rhs"].astype(np.float32)
    return att_lhs, att_rhs, res


def kernel(lhs, rhs, w, b):
    import os

    lhs = np.asarray(lhs, dtype=np.float32)
    rhs = np.asarray(rhs, dtype=np.float32)
    assert lhs.shape == (N_CORES * NB, SEQ, D) and rhs.shape == lhs.shape, (
        f"expected ({N_CORES * NB}, {SEQ}, {D}) inputs, got {lhs.shape}/{rhs.shape}"
    )
    had = os.environ.get("BASS_NEVER_TRACE")
    os.environ["BASS_NEVER_TRACE"] = "1"
    try:
        att_lhs, att_rhs, _ = run_device(lhs, rhs, w, b, trace=False)
    finally:
        if had is None:
            os.environ.pop("BASS_NEVER_TRACE", None)
        else:
            os.environ["BASS_NEVER_TRACE"] = had
    lhs_out = np.concatenate([lhs, att_lhs], axis=2)
    rhs_out = np.concatenate([rhs, att_rhs], axis=2)
    return lhs_out, rhs_out


# revision 11
# speedup vs baseline: 1.5565x; 1.0928x over previous
"""BidafAttention Trainium2 kernel (fp8 DoubleRow version).

score[b,l,r] = tanh( (lhs*w_prod) @ rhs^T + (lhs@w_l)[:,None] + (rhs@w_r)[None,:] + b )
a_lhs = softmax_R(score); a_rhs = softmax_L(score)
lhs_out = concat([lhs, a_lhs @ rhs], -1); rhs_out = concat([rhs, a_rhs^T @ lhs], -1)

Strategy: data-parallel over batch N=16 -> 2 batches per NeuronCore.
All heavy matmuls run in fp8e4 with perf_mode=DoubleRow (two K=128
contraction tiles per instruction -> ~1.8x PE throughput vs bf16).
Quantization scheme: w_prod is sqrt-split over both score operands
(lhs' = lhs*sign(w)*sqrt|w|*8, rhs' = rhs*sqrt|w|*8) so both stay in
e4m3's normal range; the resulting 64x score scale is removed by the
tanh activation's free scale factor. v (per-r bias) enters the score
PSUM via a K=1 ones-matmul; u (per-l) rides the tanh bias input.
E = exp(tanh(.)) is written as fp8 directly with rowsum via accum_out;
E^T via PE transposes (fp8), colsum rides the E^T copies' accum_out.
Outputs are computed in fp32 PSUM, scaled by 1/rowsum (1/colsum) on
DVE during PSUM->SBUF evacuation, and DMA'd out as bf16.
"""

import sys

for _p in ("/opt/trn_rl_repo",):
    if _p not in sys.path:
        sys.path.insert(0, _p)

import numpy as np
import ml_dtypes

import concourse.tile as tile
import concourse.mybir as mybir
from concourse import bacc
from concourse.bass_utils import run_bass_kernel_spmd

AF = mybir.ActivationFunctionType
DR = mybir.MatmulPerfMode.DoubleRow
BF16 = mybir.dt.bfloat16
F32 = mybir.dt.float32
F8 = mybir.dt.float8e4

P = 128
SEQ = 1024  # L == R == D == 1024
NT = SEQ // P  # 8 tiles per dim
NKP = NT // 2  # 4 DoubleRow k-pairs
CH = 512  # psum chunk (free dim)
NCH = SEQ // CH  # 2
NB = 2  # batches per core
N_CORES = 8
D = 1024
SCL = 64.0  # score scale from the 8x-per-side fp8 quantization scaling
N_WARMUP = 8  # dataless N=512 matmuls (~3.5us cold) to lift the HAM clock gate

_nc_cache = None


def _build_program():
    nc = bacc.Bacc("TRN2", target_bir_lowering=False, debug=False, num_devices=N_CORES)

    lhs_n = nc.declare_dram_parameter("lhs_n", [NB, SEQ, D], F8, isOutput=False)
    rhs_n = nc.declare_dram_parameter("rhs_n", [NB, SEQ, D], F8, isOutput=False)
    lhs_t = nc.declare_dram_parameter("lhs_t", [NB, D, SEQ], F8, isOutput=False)
    rhs_t = nc.declare_dram_parameter("rhs_t", [NB, D, SEQ], F8, isOutput=False)
    u_d = nc.declare_dram_parameter("u", [NB, P, NT], F32, isOutput=False)
    vb_d = nc.declare_dram_parameter("vb", [NB, P, SEQ], BF16, isOutput=False)
    id_d = nc.declare_dram_parameter("id8", [P, P], F8, isOutput=False)
    att_lhs = nc.declare_dram_parameter("att_lhs", [NB, SEQ, D], BF16, isOutput=True)
    att_rhs = nc.declare_dram_parameter("att_rhs", [NB, SEQ, D], BF16, isOutput=True)

    from contextlib import ExitStack

    with tile.TileContext(nc) as tc, ExitStack() as ctx:
        const = ctx.enter_context(tc.tile_pool(name="const", bufs=1))
        ident = const.tile([P, P], F8)
        nc.sync.dma_start(ident[:], id_d[:])
        ones1 = const.tile([1, P], BF16, name="ones1")
        nc.any.memset(ones1[:], 1.0)
        onesN = const.tile([1, CH], BF16, name="onesN")
        nc.any.memset(onesN[:], 1.0)

        pool_in = ctx.enter_context(tc.tile_pool(name="inf8", bufs=2))
        pool_tr = ctx.enter_context(tc.tile_pool(name="trf8", bufs=2))
        pool_e = ctx.enter_context(tc.tile_pool(name="ef8", bufs=2))
        pool_T = ctx.enter_context(tc.tile_pool(name="tanh", bufs=4))
        pool_sm = ctx.enter_context(tc.tile_pool(name="small", bufs=2))
        pool_out = ctx.enter_context(tc.tile_pool(name="osb", bufs=4))
        pool_dram = ctx.enter_context(tc.tile_pool(name="scr", bufs=1, space="DRAM"))
        psum_s = ctx.enter_context(tc.tile_pool(name="ps_s", bufs=3, space="PSUM"))
        psum_tr = ctx.enter_context(tc.tile_pool(name="ps_tr", bufs=2, space="PSUM"))
        psum_o = ctx.enter_context(tc.tile_pool(name="ps_o", bufs=3, space="PSUM"))

        # --- PE warmup: keep TensorE busy from t=0 so the HAM clock gate
        # opens (1.2 -> 2.4 GHz) before the first real matmul arrives.
        # Dataless (memset operands only) so it needs no DMA round-trip and
        # can start right after the engine barrier.
        wps = psum_s.tile([P, CH], F32, tag="ps", name="warm_ps")
        for w in range(N_WARMUP):
            nc.tensor.matmul(
                wps[:], ones1[0:1, :], onesN[0:1, :],
                start=(w == 0), stop=(w == N_WARMUP - 1),
            )
        wsb = const.tile([P, P], F8, name="warm_sb")
        nc.scalar.copy(wsb[:], wps[:, 0:P])

        for b in range(NB):
            lhs_nat = pool_in.tile([P, NT, SEQ], F8, tag="lhs_nat", name=f"lhs_nat{b}")
            rhs_nat = pool_in.tile([P, NT, SEQ], F8, tag="rhs_nat", name=f"rhs_nat{b}")
            lhsT = pool_tr.tile([P, NT, SEQ], F8, tag="lhsT", name=f"lhsT{b}")
            rhsT = pool_tr.tile([P, NT, SEQ], F8, tag="rhsT", name=f"rhsT{b}")
            u_sb = pool_sm.tile([P, NT], F32, tag="u", name=f"u{b}")
            nc.sync.dma_start(u_sb[:], u_d[b])
            vb_sb = pool_sm.tile([P, SEQ], BF16, tag="vb", name=f"vb{b}")
            nc.sync.dma_start(vb_sb[:], vb_d[b])
            # transposed (d-major) operands first: the score matmuls need
            # them. One descriptor per column-half, l/r half 0 first so the
            # jc=0 score chunks unblock as early as possible.
            for half in range(2):
                sl = slice(half * CH, (half + 1) * CH)
                nc.sync.dma_start(
                    lhsT[:, :, sl],
                    lhs_t[b].rearrange("(k p) x -> p k x", p=P)[:, :, sl],
                )
                nc.sync.dma_start(
                    rhsT[:, :, sl],
                    rhs_t[b].rearrange("(k p) x -> p k x", p=P)[:, :, sl],
                )
            nc.sync.dma_start(
                lhs_nat[:, :, :], lhs_n[b].rearrange("(k p) x -> p k x", p=P)
            )
            nc.sync.dma_start(
                rhs_nat[:, :, :], rhs_n[b].rearrange("(k p) x -> p k x", p=P)
            )

            E = pool_e.tile([P, NT, SEQ], F8, tag="E", name=f"E{b}")
            E_T = pool_e.tile([P, NT, SEQ], F8, tag="E_T", name=f"E_T{b}")
            rparts = pool_sm.tile([P, 2, NT], F32, tag="rparts", name=f"rparts{b}")
            cparts = pool_sm.tile([P, 2, NT], F32, tag="cparts", name=f"cparts{b}")

            r_col = pool_sm.tile([P, NT], F32, tag="rcol", name=f"rcol{b}")

            def score_chunk(jc, i):
                S_ps = psum_s.tile([P, CH], F32, tag="ps", name=f"S{b}_{i}_{jc}")
                for kp in range(NKP):
                    nc.tensor.matmul(
                        S_ps[:],
                        lhsT[:, 2 * kp:2 * kp + 2, i * P:(i + 1) * P],
                        rhsT[:, 2 * kp:2 * kp + 2, jc * CH:(jc + 1) * CH],
                        start=(kp == 0),
                        stop=(kp == NKP - 1),
                        perf_mode=DR,
                    )
                # += v[r] on DVE (keeps TensorE for real matmuls)
                nc.vector.tensor_add(
                    S_ps[:], S_ps[:], vb_sb[:, jc * CH:(jc + 1) * CH]
                )
                T_scr = pool_T.tile([P, CH], F32, tag="T", name=f"T{b}_{i}_{jc}")
                nc.scalar.activation(
                    T_scr[:],
                    S_ps[:],
                    AF.Tanh,
                    bias=u_sb[:, i:i + 1],
                    scale=1.0 / SCL,
                )
                nc.scalar.activation(
                    E[:, i, jc * CH:(jc + 1) * CH],
                    T_scr[:],
                    AF.Exp,
                    accum_out=rparts[:, jc, i:i + 1],
                )

            # att_rhs[r, d] = (1/colsum[r]) * sum_l E[l,r] lhs[l,d]
            # 4-transpose bursts alternate with the matmul groups; colsum
            # rides the E^T copies' accum_out.
            def att_rhs_j(j):
                po_tiles = []
                for half in range(2):
                    # fp8 PE transpose requires output element step 2: write
                    # plane 0 of a [P, CH, 2] psum tile, read it back strided.
                    pt = psum_tr.tile([P, CH, 2], F8, tag="ptr", name=f"pte{b}_{j}_{half}")
                    for q in range(4):
                        i = half * 4 + q
                        nc.tensor.transpose(
                            pt[:, q * P:(q + 1) * P, 0],
                            E[:, i, j * P:(j + 1) * P],
                            ident[:],
                        )
                    nc.scalar.activation(
                        E_T[:, j, half * CH:(half + 1) * CH],
                        pt[:, :, 0],
                        AF.Copy,
                        accum_out=cparts[:, half, j:j + 1],
                    )
                    dc = half
                    po = psum_o.tile([P, CH], F32, tag="po", name=f"por{b}_{j}_{dc}")
                    for kp in range(NKP):
                        nc.tensor.matmul(
                            po[:],
                            E[:, 2 * kp:2 * kp + 2, j * P:(j + 1) * P],
                            lhs_nat[:, 2 * kp:2 * kp + 2, dc * CH:(dc + 1) * CH],
                            start=(kp == 0),
                            stop=(kp == NKP - 1),
                            perf_mode=DR,
                        )
                    po_tiles.append(po)
                nc.vector.tensor_add(
                    r_col[:, j:j + 1], cparts[:, 0, j:j + 1], cparts[:, 1, j:j + 1]
                )
                nc.vector.reciprocal(r_col[:, j:j + 1], r_col[:, j:j + 1])
                osb = pool_out.tile([P, SEQ], BF16, tag="osb", name=f"or{b}_{j}")
                for dc in range(NCH):
                    nc.vector.tensor_scalar_mul(
                        osb[:, dc * CH:(dc + 1) * CH], po_tiles[dc][:], r_col[:, j:j + 1]
                    )
                nc.sync.dma_start(att_rhs[b, j * P:(j + 1) * P, :], osb[:])

            # --- score: S = (lhs')^T @ rhs' + v, T = tanh(S/64 + u),
            # E = exp(T) with rowsum partials. The jc=1 pass interleaves the
            # r-half-0 att_rhs columns (their E is complete after jc=0) so
            # the PE never drains ahead of the ACT tanh/exp chain at the
            # score->att boundary.
            for i in range(NT):
                score_chunk(0, i)
            for i in range(NT):
                score_chunk(1, i)
                if i % 2 == 1:
                    att_rhs_j(i // 2)

            rowsum = pool_sm.tile([P, NT], F32, tag="rowsum", name=f"rowsum{b}")
            nc.vector.tensor_add(rowsum[:], rparts[:, 0, :], rparts[:, 1, :])
            r_row = pool_sm.tile([P, NT], F32, tag="rrow", name=f"rrow{b}")
            nc.vector.reciprocal(r_row[:], rowsum[:])

            for j in range(4, NT):
                att_rhs_j(j)

            # att_lhs[l, d] = (1/rowsum[l]) * sum_r E[l,r] rhs[r,d]
            for i in range(NT):
                osb = pool_out.tile([P, SEQ], BF16, tag="osb", name=f"ol{b}_{i}")
                for dc in range(NCH):
                    po = psum_o.tile([P, CH], F32, tag="po", name=f"pol{b}_{i}_{dc}")
                    for kp in range(NKP):
                        nc.tensor.matmul(
                            po[:],
                            E_T[:, 2 * kp:2 * kp + 2, i * P:(i + 1) * P],
                            rhs_nat[:, 2 * kp:2 * kp + 2, dc * CH:(dc + 1) * CH],
                            start=(kp == 0),
                            stop=(kp == NKP - 1),
                            perf_mode=DR,
                        )
                    nc.vector.tensor_scalar_mul(
                        osb[:, dc * CH:(dc + 1) * CH], po[:], r_row[:, i:i + 1]
                    )
                nc.sync.dma_start(att_lhs[b, i * P:(i + 1) * P, :], osb[:])

        # warmup sink: a DRAM write keeps the warmup chain live; emitted
        # last so no real DMA ever queues behind the warmup dependency
        warm_dram = pool_dram.tile([P, P], F8, tag="warm", name="warm_dram")
        nc.sync.dma_start(warm_dram[:], wsb[:])

    nc.compile()
    return nc


def _get_nc():
    global _nc_cache
    if _nc_cache is None:
        _nc_cache = _build_program()
    return _nc_cache


def _prepare_in_maps(lhs, rhs, w, b):
    lhs = np.ascontiguousarray(lhs, dtype=np.float32)
    rhs = np.ascontiguousarray(rhs, dtype=np.float32)
    w = np.asarray(w, dtype=np.float32)
    b = np.float32(b)
    w_prod, w_l, w_r = w[:D], w[D:2 * D], w[2 * D:]

    # tiny host matvecs (exact, fp32)
    u_full = lhs @ w_l + b  # (N, L)
    v_full = rhs @ w_r      # (N, R)

    f8 = ml_dtypes.float8_e4m3
    bf = ml_dtypes.bfloat16
    id8 = np.eye(P, dtype=f8)
    lhs_n8 = lhs.astype(f8)
    rhs_n8 = rhs.astype(f8)
    # d-major score operands; w_prod sqrt-split over both sides, 8x
    # per-side scale keeps e4m3 operands in the normal range.
    sq = np.sqrt(np.abs(w_prod))
    lhs_t8 = np.ascontiguousarray(
        (lhs * (np.sign(w_prod) * sq * 8.0)).transpose(0, 2, 1)
    ).astype(f8)
    rhs_t8 = np.ascontiguousarray((rhs * (sq * 8.0)).transpose(0, 2, 1)).astype(f8)

    in_maps = []
    for c in range(N_CORES):
        b0 = c * NB
        u_arr = np.ascontiguousarray(
            u_full[b0:b0 + NB].reshape(NB, NT, P).transpose(0, 2, 1)
        )  # (NB, 128, 8)
        v_bf = (v_full[b0:b0 + NB] * SCL).astype(bf)  # (NB, R)
        vb_arr = np.ascontiguousarray(
            np.broadcast_to(v_bf[:, None, :], (NB, P, SEQ))
        )
        in_maps.append(
            {
                "lhs_n": lhs_n8[b0:b0 + NB],
                "rhs_n": rhs_n8[b0:b0 + NB],
                "lhs_t": lhs_t8[b0:b0 + NB],
                "rhs_t": rhs_t8[b0:b0 + NB],
                "u": u_arr,
                "vb": vb_arr,
                "id8": id8,
            }
        )
    return in_maps


def run_device(lhs, rhs, w, b, trace=False):
    """Returns (att_lhs, att_rhs, BassKernelResults)."""
    nc = _get_nc()
    in_maps = _prepare_in_maps(lhs, rhs, w, b)
    res = run_bass_kernel_spmd(
        nc, in_maps, core_ids=list(range(N_CORES)), trace=trace
    )
    N = lhs.shape[0]
    att_lhs = np.empty((N, SEQ, D), dtype=np.float32)
    att_rhs = np.empty((N, SEQ, D), dtype=np.float32)
    for c in range(N_CORES):
        b0 = c * NB
        att_lhs[b0:b0 + NB] = res.results[c]["att_lhs"].astype(np.float32)
        att_rhs[b0:b0 + NB] = res.results[c]["att_rhs"].astype(np.float32)
    return att_lhs, att_rhs, res


def kernel(lhs, rhs, w, b):
    import os

    lhs = np.asarray(lhs, dtype=np.float32)
    rhs = np.asarray(rhs, dtype=np.float32)
    assert lhs.shape == (N_CORES * NB, SEQ, D) and rhs.shape == lhs.shape, (
        f"expected ({N_CORES * NB}, {SEQ}, {D}) inputs, got {lhs.shape}/{rhs.shape}"
    )
    had = os.environ.get("BASS_NEVER_TRACE")
    os.environ["BASS_NEVER_TRACE"] = "1"
    try:
        att_lhs, att_rhs, _ = run_device(lhs, rhs, w, b, trace=False)
    finally:
        if had is None:
            os.environ.pop("BASS_NEVER_TRACE", None)
        else:
            os.environ["BASS_NEVER_TRACE"] = had
    lhs_out = np.concatenate([lhs, att_lhs], axis=2)
    rhs_out = np.concatenate([rhs, att_rhs], axis=2)
    return lhs_out, rhs_out


# revision 16
# speedup vs baseline: 1.5784x; 1.0141x over previous
"""BidafAttention Trainium2 kernel (fp8 DoubleRow version).

score[b,l,r] = tanh( (lhs*w_prod) @ rhs^T + (lhs@w_l)[:,None] + (rhs@w_r)[None,:] + b )
a_lhs = softmax_R(score); a_rhs = softmax_L(score)
lhs_out = concat([lhs, a_lhs @ rhs], -1); rhs_out = concat([rhs, a_rhs^T @ lhs], -1)

Strategy: data-parallel over batch N=16 -> 2 batches per NeuronCore.
All heavy matmuls run in fp8e4 with perf_mode=DoubleRow (two K=128
contraction tiles per instruction -> ~1.8x PE throughput vs bf16).
Quantization scheme: w_prod is sqrt-split over both score operands
(lhs' = lhs*sign(w)*sqrt|w|*8, rhs' = rhs*sqrt|w|*8) so both stay in
e4m3's normal range; the resulting 64x score scale is removed by the
tanh activation's free scale factor. v (per-r bias) enters the score
PSUM via a K=1 ones-matmul; u (per-l) rides the tanh bias input.
E = exp(tanh(.)) is written as fp8 directly with rowsum via accum_out;
E^T via PE transposes (fp8), colsum rides the E^T copies' accum_out.
Outputs are computed in fp32 PSUM, scaled by 1/rowsum (1/colsum) on
DVE during PSUM->SBUF evacuation, and DMA'd out as bf16.
"""

import sys

for _p in ("/opt/trn_rl_repo",):
    if _p not in sys.path:
        sys.path.insert(0, _p)

import numpy as np
import ml_dtypes

import concourse.tile as tile
import concourse.mybir as mybir
from concourse import bacc
from concourse.bass_utils import run_bass_kernel_spmd

AF = mybir.ActivationFunctionType
DR = mybir.MatmulPerfMode.DoubleRow
BF16 = mybir.dt.bfloat16
F32 = mybir.dt.float32
F8 = mybir.dt.float8e4

P = 128
SEQ = 1024  # L == R == D == 1024
NT = SEQ // P  # 8 tiles per dim
NKP = NT // 2  # 4 DoubleRow k-pairs
CH = 512  # psum chunk (free dim)
NCH = SEQ // CH  # 2
NB = 2  # batches per core
N_CORES = 8
D = 1024
SCL = 64.0  # score scale from the 8x-per-side fp8 quantization scaling
N_WARMUP = 9  # dataless N=512 matmuls (~3.9us cold) to lift the HAM clock gate

_nc_cache = None


def _build_program():
    nc = bacc.Bacc("TRN2", target_bir_lowering=False, debug=False, num_devices=N_CORES)

    lhs_n = nc.declare_dram_parameter("lhs_n", [NB, SEQ, D], F8, isOutput=False)
    rhs_n = nc.declare_dram_parameter("rhs_n", [NB, SEQ, D], F8, isOutput=False)
    lhs_t = nc.declare_dram_parameter("lhs_t", [NB, D, SEQ], F8, isOutput=False)
    rhs_t = nc.declare_dram_parameter("rhs_t", [NB, D, SEQ], F8, isOutput=False)
    u_d = nc.declare_dram_parameter("u", [NB, P, NT], F32, isOutput=False)
    vb_d = nc.declare_dram_parameter("vb", [NB, P, SEQ], BF16, isOutput=False)
    id_d = nc.declare_dram_parameter("id8", [P, P], F8, isOutput=False)
    att_lhs = nc.declare_dram_parameter("att_lhs", [NB, SEQ, D], BF16, isOutput=True)
    att_rhs = nc.declare_dram_parameter("att_rhs", [NB, SEQ, D], BF16, isOutput=True)

    from contextlib import ExitStack

    with tile.TileContext(nc) as tc, ExitStack() as ctx:
        const = ctx.enter_context(tc.tile_pool(name="const", bufs=1))
        ident = const.tile([P, P], F8)
        nc.sync.dma_start(ident[:], id_d[:])
        ones1 = const.tile([1, P], BF16, name="ones1")
        nc.any.memset(ones1[:], 1.0)
        onesN = const.tile([1, CH], BF16, name="onesN")
        nc.any.memset(onesN[:], 1.0)

        pool_in = ctx.enter_context(tc.tile_pool(name="inf8", bufs=2))
        pool_tr = ctx.enter_context(tc.tile_pool(name="trf8", bufs=2))
        pool_e = ctx.enter_context(tc.tile_pool(name="ef8", bufs=2))
        pool_T = ctx.enter_context(tc.tile_pool(name="tanh", bufs=4))
        pool_sm = ctx.enter_context(tc.tile_pool(name="small", bufs=2))
        pool_out = ctx.enter_context(tc.tile_pool(name="osb", bufs=4))
        pool_dram = ctx.enter_context(tc.tile_pool(name="scr", bufs=1, space="DRAM"))
        psum_s = ctx.enter_context(tc.tile_pool(name="ps_s", bufs=3, space="PSUM"))
        psum_tr = ctx.enter_context(tc.tile_pool(name="ps_tr", bufs=2, space="PSUM"))
        psum_o = ctx.enter_context(tc.tile_pool(name="ps_o", bufs=3, space="PSUM"))

        # --- PE warmup: keep TensorE busy from t=0 so the HAM clock gate
        # opens (1.2 -> 2.4 GHz) before the first real matmul arrives.
        # Dataless (memset operands only) so it needs no DMA round-trip and
        # can start right after the engine barrier.
        wps = psum_s.tile([P, CH], F32, tag="ps", name="warm_ps")
        for w in range(N_WARMUP):
            nc.tensor.matmul(
                wps[:], ones1[0:1, :], onesN[0:1, :],
                start=(w == 0), stop=(w == N_WARMUP - 1),
            )
        wsb = const.tile([P, P], F8, name="warm_sb")
        nc.scalar.copy(wsb[:], wps[:, 0:P])

        for b in range(NB):
            lhs_nat = pool_in.tile([P, NT, SEQ], F8, tag="lhs_nat", name=f"lhs_nat{b}")
            rhs_nat = pool_in.tile([P, NT, SEQ], F8, tag="rhs_nat", name=f"rhs_nat{b}")
            lhsT = pool_tr.tile([P, NT, SEQ], F8, tag="lhsT", name=f"lhsT{b}")
            rhsT = pool_tr.tile([P, NT, SEQ], F8, tag="rhsT", name=f"rhsT{b}")
            u_sb = pool_sm.tile([P, NT], F32, tag="u", name=f"u{b}")
            vb_sb = pool_sm.tile([P, SEQ], BF16, tag="vb", name=f"vb{b}")
            # transposed (d-major) operands first: the score matmuls need
            # them. One descriptor per column-half, l/r half 0 first so the
            # jc=0 score chunks unblock as early as possible; the small
            # u/vb bias loads follow (needed ~1us into the score phase).
            h0 = slice(0, CH)
            nc.sync.dma_start(
                lhsT[:, :, h0], lhs_t[b].rearrange("(k p) x -> p k x", p=P)[:, :, h0]
            )
            nc.sync.dma_start(
                rhsT[:, :, h0], rhs_t[b].rearrange("(k p) x -> p k x", p=P)[:, :, h0]
            )
            nc.sync.dma_start(u_sb[:], u_d[b])
            nc.sync.dma_start(vb_sb[:], vb_d[b])
            h1 = slice(CH, SEQ)
            nc.sync.dma_start(
                lhsT[:, :, h1], lhs_t[b].rearrange("(k p) x -> p k x", p=P)[:, :, h1]
            )
            nc.sync.dma_start(
                rhsT[:, :, h1], rhs_t[b].rearrange("(k p) x -> p k x", p=P)[:, :, h1]
            )
            nc.sync.dma_start(
                lhs_nat[:, :, :], lhs_n[b].rearrange("(k p) x -> p k x", p=P)
            )
            nc.sync.dma_start(
                rhs_nat[:, :, :], rhs_n[b].rearrange("(k p) x -> p k x", p=P)
            )

            E = pool_e.tile([P, NT, SEQ], F8, tag="E", name=f"E{b}")
            E_T = pool_e.tile([P, NT, SEQ], F8, tag="E_T", name=f"E_T{b}")
            rparts = pool_sm.tile([P, 2, NT], F32, tag="rparts", name=f"rparts{b}")
            cparts = pool_sm.tile([P, 2, NT], F32, tag="cparts", name=f"cparts{b}")

            r_col = pool_sm.tile([P, NT], F32, tag="rcol", name=f"rcol{b}")

            # exp is deferred EXP_LAG chunks behind tanh so the S psum bank
            # recycles at tanh rate (~0.9us/chunk, matching the PE's 4-DR-MM
            # rate) instead of tanh+exp rate; the lagging exps drain into the
            # ACT slack of the att_rhs phase.
            EXP_LAG = 2
            pend_exp = []

            def flush_exp():
                jc, i, T_scr = pend_exp.pop(0)
                nc.scalar.activation(
                    E[:, i, jc * CH:(jc + 1) * CH],
                    T_scr[:],
                    AF.Exp,
                    accum_out=rparts[:, jc, i:i + 1],
                )

            def score_chunk(jc, i):
                S_ps = psum_s.tile([P, CH], F32, tag="ps", name=f"S{b}_{i}_{jc}")
                for kp in range(NKP):
                    nc.tensor.matmul(
                        S_ps[:],
                        lhsT[:, 2 * kp:2 * kp + 2, i * P:(i + 1) * P],
                        rhsT[:, 2 * kp:2 * kp + 2, jc * CH:(jc + 1) * CH],
                        start=(kp == 0),
                        stop=(kp == NKP - 1),
                        perf_mode=DR,
                    )
                # += v[r] on DVE (keeps TensorE for real matmuls)
                nc.vector.tensor_add(
                    S_ps[:], S_ps[:], vb_sb[:, jc * CH:(jc + 1) * CH]
                )
                T_scr = pool_T.tile([P, CH], F32, tag="T", name=f"T{b}_{i}_{jc}")
                nc.scalar.activation(
                    T_scr[:],
                    S_ps[:],
                    AF.Tanh,
                    bias=u_sb[:, i:i + 1],
                    scale=1.0 / SCL,
                )
                pend_exp.append((jc, i, T_scr))
                if len(pend_exp) > EXP_LAG:
                    flush_exp()

            # att_rhs[r, d] = (1/colsum[r]) * sum_l E[l,r] lhs[l,d]
            # 4-transpose bursts alternate with the matmul groups; colsum
            # rides the E^T copies' accum_out.
            def att_rhs_j(j):
                po_tiles = []
                for half in range(2):
                    # fp8 PE transpose requires output element step 2: write
                    # plane 0 of a [P, CH, 2] psum tile, read it back strided.
                    pt = psum_tr.tile([P, CH, 2], F8, tag="ptr", name=f"pte{b}_{j}_{half}")
                    for q in range(4):
                        i = half * 4 + q
                        nc.tensor.transpose(
                            pt[:, q * P:(q + 1) * P, 0],
                            E[:, i, j * P:(j + 1) * P],
                            ident[:],
                        )
                    nc.scalar.activation(
                        E_T[:, j, half * CH:(half + 1) * CH],
                        pt[:, :, 0],
                        AF.Copy,
                        accum_out=cparts[:, half, j:j + 1],
                    )
                    dc = half
                    po = psum_o.tile([P, CH], F32, tag="po", name=f"por{b}_{j}_{dc}")
                    for kp in range(NKP):
                        nc.tensor.matmul(
                            po[:],
                            E[:, 2 * kp:2 * kp + 2, j * P:(j + 1) * P],
                            lhs_nat[:, 2 * kp:2 * kp + 2, dc * CH:(dc + 1) * CH],
                            start=(kp == 0),
                            stop=(kp == NKP - 1),
                            perf_mode=DR,
                        )
                    po_tiles.append(po)
                nc.vector.tensor_add(
                    r_col[:, j:j + 1], cparts[:, 0, j:j + 1], cparts[:, 1, j:j + 1]
                )
                nc.vector.reciprocal(r_col[:, j:j + 1], r_col[:, j:j + 1])
                osb = pool_out.tile([P, SEQ], BF16, tag="osb", name=f"or{b}_{j}")
                for dc in range(NCH):
                    nc.vector.tensor_scalar_mul(
                        osb[:, dc * CH:(dc + 1) * CH], po_tiles[dc][:], r_col[:, j:j + 1]
                    )
                nc.sync.dma_start(att_rhs[b, j * P:(j + 1) * P, :], osb[:])

            # --- score: S = (lhs')^T @ rhs' + v, T = tanh(S/64 + u),
            # E = exp(T) with rowsum partials. The jc=1 pass interleaves the
            # r-half-0 att_rhs columns (their E is complete after jc=0) so
            # the PE never drains ahead of the ACT tanh/exp chain at the
            # score->att boundary.
            for i in range(NT):
                score_chunk(0, i)
            for i in range(NT):
                score_chunk(1, i)
                if i >= 3 and i % 2 == 1:
                    att_rhs_j((i - 3) // 2)
            while pend_exp:
                flush_exp()
            att_rhs_j(3)

            rowsum = pool_sm.tile([P, NT], F32, tag="rowsum", name=f"rowsum{b}")
            nc.vector.tensor_add(rowsum[:], rparts[:, 0, :], rparts[:, 1, :])
            r_row = pool_sm.tile([P, NT], F32, tag="rrow", name=f"rrow{b}")
            nc.vector.reciprocal(r_row[:], rowsum[:])

            for j in range(4, NT):
                att_rhs_j(j)

            # att_lhs[l, d] = (1/rowsum[l]) * sum_r E[l,r] rhs[r,d]
            # (output DMA per d-half so the dc=0 half ships while dc=1
            # computes -- shortens the kernel tail)
            for i in range(NT):
                osb = pool_out.tile([P, SEQ], BF16, tag="osb", name=f"ol{b}_{i}")
                for dc in range(NCH):
                    po = psum_o.tile([P, CH], F32, tag="po", name=f"pol{b}_{i}_{dc}")
                    for kp in range(NKP):
                        nc.tensor.matmul(
                            po[:],
                            E_T[:, 2 * kp:2 * kp + 2, i * P:(i + 1) * P],
                            rhs_nat[:, 2 * kp:2 * kp + 2, dc * CH:(dc + 1) * CH],
                            start=(kp == 0),
                            stop=(kp == NKP - 1),
                            perf_mode=DR,
                        )
                    nc.vector.tensor_scalar_mul(
                        osb[:, dc * CH:(dc + 1) * CH], po[:], r_row[:, i:i + 1]
                    )
                    nc.sync.dma_start(
                        att_lhs[b, i * P:(i + 1) * P, dc * CH:(dc + 1) * CH],
                        osb[:, dc * CH:(dc + 1) * CH],
                    )

        # warmup sink: a DRAM write keeps the warmup chain live; emitted
        # last so no real DMA ever queues behind the warmup dependency
        warm_dram = pool_dram.tile([P, P], F8, tag="warm", name="warm_dram")
        nc.sync.dma_start(warm_dram[:], wsb[:])

    nc.compile()
    return nc


def _get_nc():
    global _nc_cache
    if _nc_cache is None:
        _nc_cache = _build_program()
    return _nc_cache


def _prepare_in_maps(lhs, rhs, w, b):
    lhs = np.ascontiguousarray(lhs, dtype=np.float32)
    rhs = np.ascontiguousarray(rhs, dtype=np.float32)
    w = np.asarray(w, dtype=np.float32)
    b = np.float32(b)
    w_prod, w_l, w_r = w[:D], w[D:2 * D], w[2 * D:]

    # tiny host matvecs (exact, fp32)
    u_full = lhs @ w_l + b  # (N, L)
    v_full = rhs @ w_r      # (N, R)

    f8 = ml_dtypes.float8_e4m3
    bf = ml_dtypes.bfloat16
    id8 = np.eye(P, dtype=f8)
    lhs_n8 = lhs.astype(f8)
    rhs_n8 = rhs.astype(f8)
    # d-major score operands; w_prod sqrt-split over both sides, 8x
    # per-side scale keeps e4m3 operands in the normal range.
    sq = np.sqrt(np.abs(w_prod))
    lhs_t8 = np.ascontiguousarray(
        (lhs * (np.sign(w_prod) * sq * 8.0)).transpose(0, 2, 1)
    ).astype(f8)
    rhs_t8 = np.ascontiguousarray((rhs * (sq * 8.0)).transpose(0, 2, 1)).astype(f8)

    in_maps = []
    for c in range(N_CORES):
        b0 = c * NB
        u_arr = np.ascontiguousarray(
            u_full[b0:b0 + NB].reshape(NB, NT, P).transpose(0, 2, 1)
        )  # (NB, 128, 8)
        v_bf = (v_full[b0:b0 + NB] * SCL).astype(bf)  # (NB, R)
        vb_arr = np.ascontiguousarray(
            np.broadcast_to(v_bf[:, None, :], (NB, P, SEQ))
        )
        in_maps.append(
            {
                "lhs_n": lhs_n8[b0:b0 + NB],
                "rhs_n": rhs_n8[b0:b0 + NB],
                "lhs_t": lhs_t8[b0:b0 + NB],
                "rhs_t": rhs_t8[b0:b0 + NB],
                "u": u_arr,
                "vb": vb_arr,
                "id8": id8,
            }
        )
    return in_maps


def run_device(lhs, rhs, w, b, trace=False):
    """Returns (att_lhs, att_rhs, BassKernelResults)."""
    nc = _get_nc()
    in_maps = _prepare_in_maps(lhs, rhs, w, b)
    res = run_bass_kernel_spmd(
        nc, in_maps, core_ids=list(range(N_CORES)), trace=trace
    )
    N = lhs.shape[0]
    att_lhs = np.empty((N, SEQ, D), dtype=np.float32)
    att_rhs = np.empty((N, SEQ, D), dtype=np.float32)
    for c in range(N_CORES):
        b0 = c * NB
        att_lhs[b0:b0 + NB] = res.results[c]["att_lhs"].astype(np.float32)
        att_rhs[b0:b0 + NB] = res.results[c]["att_rhs"].astype(np.float32)
    return att_lhs, att_rhs, res


def kernel(lhs, rhs, w, b):
    import os

    lhs = np.asarray(lhs, dtype=np.float32)
    rhs = np.asarray(rhs, dtype=np.float32)
    assert lhs.shape == (N_CORES * NB, SEQ, D) and rhs.shape == lhs.shape, (
        f"expected ({N_CORES * NB}, {SEQ}, {D}) inputs, got {lhs.shape}/{rhs.shape}"
    )
    had = os.environ.get("BASS_NEVER_TRACE")
    os.environ["BASS_NEVER_TRACE"] = "1"
    try:
        att_lhs, att_rhs, _ = run_device(lhs, rhs, w, b, trace=False)
    finally:
        if had is None:
            os.environ.pop("BASS_NEVER_TRACE", None)
        else:
            os.environ["BASS_NEVER_TRACE"] = had
    lhs_out = np.concatenate([lhs, att_lhs], axis=2)
    rhs_out = np.concatenate([rhs, att_rhs], axis=2)
    return lhs_out, rhs_out


# revision 19
# speedup vs baseline: 1.6420x; 1.0403x over previous
"""BidafAttention Trainium2 kernel (fp8 DoubleRow version).

score[b,l,r] = tanh( (lhs*w_prod) @ rhs^T + (lhs@w_l)[:,None] + (rhs@w_r)[None,:] + b )
a_lhs = softmax_R(score); a_rhs = softmax_L(score)
lhs_out = concat([lhs, a_lhs @ rhs], -1); rhs_out = concat([rhs, a_rhs^T @ lhs], -1)

Strategy: data-parallel over batch N=16 -> 2 batches per NeuronCore.
All heavy matmuls run in fp8e4 with perf_mode=DoubleRow (two K=128
contraction tiles per instruction -> ~1.8x PE throughput vs bf16).
Quantization scheme: w_prod is sqrt-split over both score operands
(lhs' = lhs*sign(w)*sqrt|w|*8, rhs' = rhs*sqrt|w|*8) so both stay in
e4m3's normal range; the resulting 64x score scale is removed by the
tanh activation's free scale factor. v (per-r bias) enters the score
PSUM via a K=1 ones-matmul; u (per-l) rides the tanh bias input.
E = exp(tanh(.)) is written as fp8 directly with rowsum via accum_out;
E^T via PE transposes (fp8), colsum rides the E^T copies' accum_out.
Outputs are computed in fp32 PSUM, scaled by 1/rowsum (1/colsum) on
DVE during PSUM->SBUF evacuation, and DMA'd out as bf16.
"""

import sys

for _p in ("/opt/trn_rl_repo",):
    if _p not in sys.path:
        sys.path.insert(0, _p)

import numpy as np
import ml_dtypes

import concourse.tile as tile
import concourse.mybir as mybir
from concourse import bacc
from concourse.bass_utils import run_bass_kernel_spmd

AF = mybir.ActivationFunctionType
DR = mybir.MatmulPerfMode.DoubleRow
BF16 = mybir.dt.bfloat16
F32 = mybir.dt.float32
F8 = mybir.dt.float8e4

P = 128
SEQ = 1024  # L == R == D == 1024
NT = SEQ // P  # 8 tiles per dim
NKP = NT // 2  # 4 DoubleRow k-pairs
CH = 512  # psum chunk (free dim)
NCH = SEQ // CH  # 2
NB = 2  # batches per core
N_CORES = 8
D = 1024
SCL = 64.0  # score scale from the 8x-per-side fp8 quantization scaling
N_WARMUP = 13  # dataless N=512 matmuls to lift the HAM clock gate and
               # bridge until the first score operands land (~13.5us)

_nc_cache = None


def _build_program():
    nc = bacc.Bacc("TRN2", target_bir_lowering=False, debug=False, num_devices=N_CORES)

    lhs_n = nc.declare_dram_parameter("lhs_n", [NB, SEQ, D], F8, isOutput=False)
    rhs_n = nc.declare_dram_parameter("rhs_n", [NB, SEQ, D], F8, isOutput=False)
    lhs_t = nc.declare_dram_parameter("lhs_t", [NB, D, SEQ], F8, isOutput=False)
    rhs_t = nc.declare_dram_parameter("rhs_t", [NB, D, SEQ], F8, isOutput=False)
    u_d = nc.declare_dram_parameter("u", [NB, P, NT], F32, isOutput=False)
    vb_d = nc.declare_dram_parameter("vb", [NB, P, SEQ], BF16, isOutput=False)
    id_d = nc.declare_dram_parameter("id8", [P, P], F8, isOutput=False)
    att_lhs = nc.declare_dram_parameter("att_lhs", [NB, SEQ, D], BF16, isOutput=True)
    att_rhs = nc.declare_dram_parameter("att_rhs", [NB, SEQ, D], BF16, isOutput=True)

    from contextlib import ExitStack

    with tile.TileContext(nc) as tc, ExitStack() as ctx:
        const = ctx.enter_context(tc.tile_pool(name="const", bufs=1))
        ident = const.tile([P, P], F8)  # DMA'd inside Batch 0's load sequence
        ones1 = const.tile([1, P], BF16, name="ones1")
        nc.any.memset(ones1[:], 1.0)
        onesN = const.tile([1, CH], BF16, name="onesN")
        nc.any.memset(onesN[:], 1.0)

        pool_in = ctx.enter_context(tc.tile_pool(name="inf8", bufs=2))
        pool_tr = ctx.enter_context(tc.tile_pool(name="trf8", bufs=2))
        pool_e = ctx.enter_context(tc.tile_pool(name="ef8", bufs=2))
        pool_T = ctx.enter_context(tc.tile_pool(name="tanh", bufs=4))
        pool_sm = ctx.enter_context(tc.tile_pool(name="small", bufs=2))
        pool_out = ctx.enter_context(tc.tile_pool(name="osb", bufs=4))
        pool_dram = ctx.enter_context(tc.tile_pool(name="scr", bufs=1, space="DRAM"))
        psum_s = ctx.enter_context(tc.tile_pool(name="ps_s", bufs=3, space="PSUM"))
        psum_tr = ctx.enter_context(tc.tile_pool(name="ps_tr", bufs=2, space="PSUM"))
        psum_o = ctx.enter_context(tc.tile_pool(name="ps_o", bufs=3, space="PSUM"))

        # --- PE warmup: keep TensorE busy from t=0 so the HAM clock gate
        # opens (1.2 -> 2.4 GHz) before the first real matmul arrives.
        # Dataless (memset operands only) so it needs no DMA round-trip and
        # can start right after the engine barrier.
        wps = psum_s.tile([P, CH], F32, tag="ps", name="warm_ps")
        for w in range(N_WARMUP):
            nc.tensor.matmul(
                wps[:], ones1[0:1, :], onesN[0:1, :],
                start=(w == 0), stop=(w == N_WARMUP - 1),
            )
        wsb = const.tile([P, P], F8, name="warm_sb")
        nc.scalar.copy(wsb[:], wps[:, 0:P])

        class Batch:
            """Per-batch tiles + emission helpers, so the two batches'
            instruction streams can be interleaved at orchestration level."""

            def __init__(self, b, first):
                self.b = b
                self.lhs_nat = pool_in.tile([P, NT, SEQ], F8, tag="lhs_nat", name=f"lhs_nat{b}")
                self.rhs_nat = pool_in.tile([P, NT, SEQ], F8, tag="rhs_nat", name=f"rhs_nat{b}")
                self.lhsT = pool_tr.tile([P, NT, SEQ], F8, tag="lhsT", name=f"lhsT{b}")
                self.rhsT = pool_tr.tile([P, NT, SEQ], F8, tag="rhsT", name=f"rhsT{b}")
                self.u_sb = pool_sm.tile([P, NT], F32, tag="u", name=f"u{b}")
                self.vb_sb = pool_sm.tile([P, SEQ], BF16, tag="vb", name=f"vb{b}")
                # transposed (d-major) operands first: the score matmuls
                # need them; l/r half 0 first so the jc=0 chunks unblock
                # earliest. The DGE ring holds 8 descriptors, so the first
                # batch squeezes ident in after its 4 score-critical loads.
                h0, h1 = slice(0, CH), slice(CH, SEQ)
                nc.sync.dma_start(
                    self.lhsT[:, :, h0],
                    lhs_t[b].rearrange("(k p) x -> p k x", p=P)[:, :, h0],
                )
                nc.sync.dma_start(
                    self.rhsT[:, :, h0],
                    rhs_t[b].rearrange("(k p) x -> p k x", p=P)[:, :, h0],
                )
                nc.sync.dma_start(self.u_sb[:], u_d[b])
                nc.sync.dma_start(self.vb_sb[:], vb_d[b])
                if first:
                    nc.sync.dma_start(ident[:], id_d[:])
                nc.sync.dma_start(
                    self.lhsT[:, :, h1],
                    lhs_t[b].rearrange("(k p) x -> p k x", p=P)[:, :, h1],
                )
                nc.sync.dma_start(
                    self.rhsT[:, :, h1],
                    rhs_t[b].rearrange("(k p) x -> p k x", p=P)[:, :, h1],
                )
                nc.sync.dma_start(
                    self.lhs_nat[:, :, :], lhs_n[b].rearrange("(k p) x -> p k x", p=P)
                )
                nc.sync.dma_start(
                    self.rhs_nat[:, :, :], rhs_n[b].rearrange("(k p) x -> p k x", p=P)
                )

                self.E = pool_e.tile([P, NT, SEQ], F8, tag="E", name=f"E{b}")
                self.E_T = pool_e.tile([P, NT, SEQ], F8, tag="E_T", name=f"E_T{b}")
                self.rparts = pool_sm.tile([P, 2, NT], F32, tag="rparts", name=f"rparts{b}")
                self.r_col = pool_sm.tile([P, NT], F32, tag="rcol", name=f"rcol{b}")
                # exp is deferred EXP_LAG chunks behind tanh so the S psum
                # bank recycles at tanh rate (~0.9us/chunk, matching the
                # PE's 4-DR-MM rate) instead of tanh+exp rate; the lagging
                # exps drain into the ACT slack of the att_rhs phase.
                self.pend_exp = []

            def flush_exp(self):
                jc, i, T_scr = self.pend_exp.pop(0)
                nc.scalar.activation(
                    self.E[:, i, jc * CH:(jc + 1) * CH],
                    T_scr[:],
                    AF.Exp,
                    accum_out=self.rparts[:, jc, i:i + 1],
                )

            def score_chunk(self, jc, i, lag=2):
                b = self.b
                S_ps = psum_s.tile([P, CH], F32, tag="ps", name=f"S{b}_{i}_{jc}")
                for kp in range(NKP):
                    nc.tensor.matmul(
                        S_ps[:],
                        self.lhsT[:, 2 * kp:2 * kp + 2, i * P:(i + 1) * P],
                        self.rhsT[:, 2 * kp:2 * kp + 2, jc * CH:(jc + 1) * CH],
                        start=(kp == 0),
                        stop=(kp == NKP - 1),
                        perf_mode=DR,
                    )
                # += v[r] on DVE (keeps TensorE for real matmuls)
                nc.vector.tensor_add(
                    S_ps[:], S_ps[:], self.vb_sb[:, jc * CH:(jc + 1) * CH]
                )
                T_scr = pool_T.tile([P, CH], F32, tag="T", name=f"T{b}_{i}_{jc}")
                nc.scalar.activation(
                    T_scr[:],
                    S_ps[:],
                    AF.Tanh,
                    bias=self.u_sb[:, i:i + 1],
                    scale=1.0 / SCL,
                )
                self.pend_exp.append((jc, i, T_scr))
                if len(self.pend_exp) > lag:
                    self.flush_exp()

            # att_rhs[r, d] = (1/colsum[r]) * sum_l E[l,r] lhs[l,d]
            # 8-transpose bursts alternate with the matmul groups; colsum
            # rides the E^T copy's accum_out.
            def att_rhs_j(self, j):
                b = self.b
                E, E_T = self.E, self.E_T
                # fp8 PE transpose requires output element step 2: all 8
                # l-tiles of column j fill one bank ([P, SEQ, 2] fp8 = 2KB),
                # evacuated by a single strided full-row copy whose
                # accum_out is colsum[j] directly.
                pt = psum_tr.tile([P, SEQ, 2], F8, tag="ptr", name=f"pte{b}_{j}")
                for i in range(NT):
                    nc.tensor.transpose(
                        pt[:, i * P:(i + 1) * P, 0],
                        E[:, i, j * P:(j + 1) * P],
                        ident[:],
                    )
                nc.scalar.activation(
                    E_T[:, j, :],
                    pt[:, :, 0],
                    AF.Copy,
                    accum_out=self.r_col[:, j:j + 1],
                )
                nc.vector.reciprocal(self.r_col[:, j:j + 1], self.r_col[:, j:j + 1])
                po_tiles = []
                for dc in range(NCH):
                    po = psum_o.tile([P, CH], F32, tag="po", name=f"por{b}_{j}_{dc}")
                    for kp in range(NKP):
                        nc.tensor.matmul(
                            po[:],
                            E[:, 2 * kp:2 * kp + 2, j * P:(j + 1) * P],
                            self.lhs_nat[:, 2 * kp:2 * kp + 2, dc * CH:(dc + 1) * CH],
                            start=(kp == 0),
                            stop=(kp == NKP - 1),
                            perf_mode=DR,
                        )
                    po_tiles.append(po)
                osb = pool_out.tile([P, SEQ], BF16, tag="osb", name=f"or{b}_{j}")
                for dc in range(NCH):
                    nc.vector.tensor_scalar_mul(
                        osb[:, dc * CH:(dc + 1) * CH], po_tiles[dc][:],
                        self.r_col[:, j:j + 1],
                    )
                nc.sync.dma_start(att_rhs[b, j * P:(j + 1) * P, :], osb[:])

            def rowsum(self):
                b = self.b
                rowsum = pool_sm.tile([P, NT], F32, tag="rowsum", name=f"rowsum{b}")
                nc.vector.tensor_add(rowsum[:], self.rparts[:, 0, :], self.rparts[:, 1, :])
                self.r_row = pool_sm.tile([P, NT], F32, tag="rrow", name=f"rrow{b}")
                nc.vector.reciprocal(self.r_row[:], rowsum[:])

            # att_lhs[l, d] = (1/rowsum[l]) * sum_r E[l,r] rhs[r,d]
            # (output DMA per d-half so the dc=0 half ships while dc=1
            # computes -- shortens the kernel tail)
            def att_lhs_i(self, i):
                b = self.b
                osb = pool_out.tile([P, SEQ], BF16, tag="osb", name=f"ol{b}_{i}")
                for dc in range(NCH):
                    po = psum_o.tile([P, CH], F32, tag="po", name=f"pol{b}_{i}_{dc}")
                    for kp in range(NKP):
                        nc.tensor.matmul(
                            po[:],
                            self.E_T[:, 2 * kp:2 * kp + 2, i * P:(i + 1) * P],
                            self.rhs_nat[:, 2 * kp:2 * kp + 2, dc * CH:(dc + 1) * CH],
                            start=(kp == 0),
                            stop=(kp == NKP - 1),
                            perf_mode=DR,
                        )
                    nc.vector.tensor_scalar_mul(
                        osb[:, dc * CH:(dc + 1) * CH], po[:], self.r_row[:, i:i + 1]
                    )
                    nc.sync.dma_start(
                        att_lhs[b, i * P:(i + 1) * P, dc * CH:(dc + 1) * CH],
                        osb[:, dc * CH:(dc + 1) * CH],
                    )

            def score_and_att_rhs(self):
                """score (jc=0 then jc=1, with the r-half-0 att_rhs columns
                interleaved into the jc=1 pass so the PE never drains ahead
                of the ACT tanh/exp chain), then the remaining att_rhs."""
                for i in range(NT):
                    self.score_chunk(0, i)
                for i in range(NT):
                    self.score_chunk(1, i)
                    if i >= 3 and i % 2 == 1:
                        self.att_rhs_j((i - 3) // 2)
                while self.pend_exp:
                    self.flush_exp()
                self.att_rhs_j(3)
                self.rowsum()
                for j in range(4, NT):
                    self.att_rhs_j(j)

        b0 = Batch(0, first=True)
        b1 = Batch(1, first=False)
        b0.score_and_att_rhs()
        # bridge the att_rhs -> att_lhs dependency stall (att_lhs needs the
        # last E^T copy) with the first two score chunks of the next batch
        b1.score_chunk(0, 0)
        b1.score_chunk(0, 1)
        for i in range(NT):
            b0.att_lhs_i(i)
        for i in range(2, NT):
            b1.score_chunk(0, i)
        for i in range(NT):
            b1.score_chunk(1, i)
            if i >= 3 and i % 2 == 1:
                b1.att_rhs_j((i - 3) // 2)
        while b1.pend_exp:
            b1.flush_exp()
        b1.att_rhs_j(3)
        b1.rowsum()
        for j in range(4, NT):
            b1.att_rhs_j(j)
        for i in range(NT):
            b1.att_lhs_i(i)

        # warmup sink: a DRAM write keeps the warmup chain live; emitted
        # last so no real DMA ever queues behind the warmup dependency
        warm_dram = pool_dram.tile([P, P], F8, tag="warm", name="warm_dram")
        nc.sync.dma_start(warm_dram[:], wsb[:])

    nc.compile()
    return nc


def _get_nc():
    global _nc_cache
    if _nc_cache is None:
        _nc_cache = _build_program()
    return _nc_cache


def _prepare_in_maps(lhs, rhs, w, b):
    lhs = np.ascontiguousarray(lhs, dtype=np.float32)
    rhs = np.ascontiguousarray(rhs, dtype=np.float32)
    w = np.asarray(w, dtype=np.float32)
    b = np.float32(b)
    w_prod, w_l, w_r = w[:D], w[D:2 * D], w[2 * D:]

    # tiny host matvecs (exact, fp32)
    u_full = lhs @ w_l + b  # (N, L)
    v_full = rhs @ w_r      # (N, R)

    f8 = ml_dtypes.float8_e4m3
    bf = ml_dtypes.bfloat16
    id8 = np.eye(P, dtype=f8)
    lhs_n8 = lhs.astype(f8)
    rhs_n8 = rhs.astype(f8)
    # d-major score operands; w_prod sqrt-split over both sides, 8x
    # per-side scale keeps e4m3 operands in the normal range.
    sq = np.sqrt(np.abs(w_prod))
    lhs_t8 = np.ascontiguousarray(
        (lhs * (np.sign(w_prod) * sq * 8.0)).transpose(0, 2, 1)
    ).astype(f8)
    rhs_t8 = np.ascontiguousarray((rhs * (sq * 8.0)).transpose(0, 2, 1)).astype(f8)

    in_maps = []
    for c in range(N_CORES):
        b0 = c * NB
        u_arr = np.ascontiguousarray(
            u_full[b0:b0 + NB].reshape(NB, NT, P).transpose(0, 2, 1)
        )  # (NB, 128, 8)
        v_bf = (v_full[b0:b0 + NB] * SCL).astype(bf)  # (NB, R)
        vb_arr = np.ascontiguousarray(
            np.broadcast_to(v_bf[:, None, :], (NB, P, SEQ))
        )
        in_maps.append(
            {
                "lhs_n": lhs_n8[b0:b0 + NB],
                "rhs_n": rhs_n8[b0:b0 + NB],
                "lhs_t": lhs_t8[b0:b0 + NB],
                "rhs_t": rhs_t8[b0:b0 + NB],
                "u": u_arr,
                "vb": vb_arr,
                "id8": id8,
            }
        )
    return in_maps


def run_device(lhs, rhs, w, b, trace=False):
    """Returns (att_lhs, att_rhs, BassKernelResults)."""
    nc = _get_nc()
    in_maps = _prepare_in_maps(lhs, rhs, w, b)
    res = run_bass_kernel_spmd(
        nc, in_maps, core_ids=list(range(N_CORES)), trace=trace
    )
    N = lhs.shape[0]
    att_lhs = np.empty((N, SEQ, D), dtype=np.float32)
    att_rhs = np.empty((N, SEQ, D), dtype=np.float32)
    for c in range(N_CORES):
        b0 = c * NB
        att_lhs[b0:b0 + NB] = res.results[c]["att_lhs"].astype(np.float32)
        att_rhs[b0:b0 + NB] = res.results[c]["att_rhs"].astype(np.float32)
    return att_lhs, att_rhs, res


def kernel(lhs, rhs, w, b):
    import os

    lhs = np.asarray(lhs, dtype=np.float32)
    rhs = np.asarray(rhs, dtype=np.float32)
    assert lhs.shape == (N_CORES * NB, SEQ, D) and rhs.shape == lhs.shape, (
        f"expected ({N_CORES * NB}, {SEQ}, {D}) inputs, got {lhs.shape}/{rhs.shape}"
    )
    had = os.environ.get("BASS_NEVER_TRACE")
    os.environ["BASS_NEVER_TRACE"] = "1"
    try:
        att_lhs, att_rhs, _ = run_device(lhs, rhs, w, b, trace=False)
    finally:
        if had is None:
            os.environ.pop("BASS_NEVER_TRACE", None)
        else:
            os.environ["BASS_NEVER_TRACE"] = had
    lhs_out = np.concatenate([lhs, att_lhs], axis=2)
    rhs_out = np.concatenate([rhs, att_rhs], axis=2)
    return lhs_out, rhs_out


# revision 25
# speedup vs baseline: 1.6473x; 1.0032x over previous
"""BidafAttention Trainium2 kernel (fp8 DoubleRow version).

score[b,l,r] = tanh( (lhs*w_prod) @ rhs^T + (lhs@w_l)[:,None] + (rhs@w_r)[None,:] + b )
a_lhs = softmax_R(score); a_rhs = softmax_L(score)
lhs_out = concat([lhs, a_lhs @ rhs], -1); rhs_out = concat([rhs, a_rhs^T @ lhs], -1)

Strategy: data-parallel over batch N=16 -> 2 batches per NeuronCore.
All heavy matmuls run in fp8e4 with perf_mode=DoubleRow (two K=128
contraction tiles per instruction -> ~1.8x PE throughput vs bf16).
Quantization scheme: w_prod is sqrt-split over both score operands
(lhs' = lhs*sign(w)*sqrt|w|*8, rhs' = rhs*sqrt|w|*8) so both stay in
e4m3's normal range; the resulting 64x score scale is removed by the
tanh activation's free scale factor. v (per-r bias) enters the score
PSUM via a K=1 ones-matmul; u (per-l) rides the tanh bias input.
E = exp(tanh(.)) is written as fp8 directly with rowsum via accum_out;
E^T via PE transposes (fp8), colsum rides the E^T copies' accum_out.
Outputs are computed in fp32 PSUM, scaled by 1/rowsum (1/colsum) on
DVE during PSUM->SBUF evacuation, and DMA'd out as bf16.
"""

import sys

for _p in ("/opt/trn_rl_repo",):
    if _p not in sys.path:
        sys.path.insert(0, _p)

import numpy as np
import ml_dtypes

import concourse.tile as tile
import concourse.mybir as mybir
from concourse import bacc
from concourse.bass_utils import run_bass_kernel_spmd

AF = mybir.ActivationFunctionType
DR = mybir.MatmulPerfMode.DoubleRow
BF16 = mybir.dt.bfloat16
F32 = mybir.dt.float32
F8 = mybir.dt.float8e4

P = 128
SEQ = 1024  # L == R == D == 1024
NT = SEQ // P  # 8 tiles per dim
NKP = NT // 2  # 4 DoubleRow k-pairs
CH = 512  # psum chunk (free dim)
NCH = SEQ // CH  # 2
NB = 2  # batches per core
N_CORES = 8
D = 1024
SCL = 64.0  # score scale from the 8x-per-side fp8 quantization scaling
N_WARMUP = 13  # dataless N=512 matmuls to lift the HAM clock gate and
               # bridge until the first score operands land (~13.5us)

_nc_cache = None


def _build_program():
    nc = bacc.Bacc("TRN2", target_bir_lowering=False, debug=False, num_devices=N_CORES)

    lhs_n = nc.declare_dram_parameter("lhs_n", [NB, SEQ, D], F8, isOutput=False)
    rhs_n = nc.declare_dram_parameter("rhs_n", [NB, SEQ, D], F8, isOutput=False)
    lhs_t = nc.declare_dram_parameter("lhs_t", [NB, D, SEQ], F8, isOutput=False)
    rhs_t = nc.declare_dram_parameter("rhs_t", [NB, D, SEQ], F8, isOutput=False)
    u_d = nc.declare_dram_parameter("u", [NB, P, NT], F32, isOutput=False)
    vb_d = nc.declare_dram_parameter("vb", [NB, P, SEQ], BF16, isOutput=False)
    id_d = nc.declare_dram_parameter("id8", [P, P], F8, isOutput=False)
    att_lhs = nc.declare_dram_parameter("att_lhs", [NB, SEQ, D], BF16, isOutput=True)
    att_rhs = nc.declare_dram_parameter("att_rhs", [NB, SEQ, D], BF16, isOutput=True)

    from contextlib import ExitStack

    with tile.TileContext(nc) as tc, ExitStack() as ctx:
        const = ctx.enter_context(tc.tile_pool(name="const", bufs=1))
        ident = const.tile([P, P], F8)  # DMA'd inside Batch 0's load sequence
        # full-K warmup operands: the HAM activity monitor tracks PE array
        # occupancy, so warmup matmuls must use all 128 rows to count.
        wkk = const.tile([P, P], BF16, name="wkk")
        nc.any.memset(wkk[:], 1.0)
        wmv = const.tile([P, CH], BF16, name="wmv")
        nc.any.memset(wmv[:], 1.0)

        pool_in = ctx.enter_context(tc.tile_pool(name="inf8", bufs=2))
        pool_tr = ctx.enter_context(tc.tile_pool(name="trf8", bufs=2))
        pool_e = ctx.enter_context(tc.tile_pool(name="ef8", bufs=2))
        pool_T = ctx.enter_context(tc.tile_pool(name="tanh", bufs=4))
        pool_sm = ctx.enter_context(tc.tile_pool(name="small", bufs=2))
        pool_out = ctx.enter_context(tc.tile_pool(name="osb", bufs=4))
        pool_dram = ctx.enter_context(tc.tile_pool(name="scr", bufs=1, space="DRAM"))
        psum_s = ctx.enter_context(tc.tile_pool(name="ps_s", bufs=3, space="PSUM"))
        psum_tr = ctx.enter_context(tc.tile_pool(name="ps_tr", bufs=2, space="PSUM"))
        psum_o = ctx.enter_context(tc.tile_pool(name="ps_o", bufs=3, space="PSUM"))

        # --- PE warmup: keep TensorE busy from t=0 so the HAM clock gate
        # opens (1.2 -> 2.4 GHz) before the first real matmul arrives.
        # Dataless (memset operands only) so it needs no DMA round-trip and
        # can start right after the engine barrier.
        wps = psum_s.tile([P, CH], F32, tag="ps", name="warm_ps")
        for w in range(N_WARMUP):
            nc.tensor.matmul(
                wps[:], wkk[:], wmv[:],
                start=(w == 0), stop=(w == N_WARMUP - 1),
            )
        wsb = const.tile([P, P], F8, name="warm_sb")
        nc.scalar.copy(wsb[:], wps[:, 0:P])

        class Batch:
            """Per-batch tiles + emission helpers, so the two batches'
            instruction streams can be interleaved at orchestration level."""

            def __init__(self, b, first):
                self.b = b
                self.lhs_nat = pool_in.tile([P, NT, SEQ], F8, tag="lhs_nat", name=f"lhs_nat{b}")
                self.rhs_nat = pool_in.tile([P, NT, SEQ], F8, tag="rhs_nat", name=f"rhs_nat{b}")
                self.lhsT = pool_tr.tile([P, NT, SEQ], F8, tag="lhsT", name=f"lhsT{b}")
                self.rhsT = pool_tr.tile([P, NT, SEQ], F8, tag="rhsT", name=f"rhsT{b}")
                self.u_sb = pool_sm.tile([P, NT], F32, tag="u", name=f"u{b}")
                self.vb_sb = pool_sm.tile([P, SEQ], BF16, tag="vb", name=f"vb{b}")
                # transposed (d-major) operands first: the score matmuls
                # need them; l/r half 0 first so the jc=0 chunks unblock
                # earliest. The DGE ring holds 8 descriptors, so the first
                # batch squeezes ident in after its 4 score-critical loads.
                h0, h1 = slice(0, CH), slice(CH, SEQ)
                nc.sync.dma_start(
                    self.lhsT[:, :, h0],
                    lhs_t[b].rearrange("(k p) x -> p k x", p=P)[:, :, h0],
                )
                nc.sync.dma_start(
                    self.rhsT[:, :, h0],
                    rhs_t[b].rearrange("(k p) x -> p k x", p=P)[:, :, h0],
                )
                nc.sync.dma_start(self.u_sb[:], u_d[b])
                nc.sync.dma_start(self.vb_sb[:], vb_d[b])
                if first:
                    nc.sync.dma_start(ident[:], id_d[:])
                nc.sync.dma_start(
                    self.lhsT[:, :, h1],
                    lhs_t[b].rearrange("(k p) x -> p k x", p=P)[:, :, h1],
                )
                nc.sync.dma_start(
                    self.rhsT[:, :, h1],
                    rhs_t[b].rearrange("(k p) x -> p k x", p=P)[:, :, h1],
                )
                nc.sync.dma_start(
                    self.lhs_nat[:, :, :], lhs_n[b].rearrange("(k p) x -> p k x", p=P)
                )
                nc.sync.dma_start(
                    self.rhs_nat[:, :, :], rhs_n[b].rearrange("(k p) x -> p k x", p=P)
                )

                self.E = pool_e.tile([P, NT, SEQ], F8, tag="E", name=f"E{b}")
                self.E_T = pool_e.tile([P, NT, SEQ], F8, tag="E_T", name=f"E_T{b}")
                self.rparts = pool_sm.tile([P, 2, NT], F32, tag="rparts", name=f"rparts{b}")
                self.r_col = pool_sm.tile([P, NT], F32, tag="rcol", name=f"rcol{b}")
                # exp is deferred EXP_LAG chunks behind tanh so the S psum
                # bank recycles at tanh rate (~0.9us/chunk, matching the
                # PE's 4-DR-MM rate) instead of tanh+exp rate; the lagging
                # exps drain into the ACT slack of the att_rhs phase.
                self.pend_exp = []

            def flush_exp(self):
                jc, i, T_scr = self.pend_exp.pop(0)
                nc.scalar.activation(
                    self.E[:, i, jc * CH:(jc + 1) * CH],
                    T_scr[:],
                    AF.Exp,
                    accum_out=self.rparts[:, jc, i:i + 1],
                )

            def score_chunk(self, jc, i, lag=0):
                b = self.b
                S_ps = psum_s.tile([P, CH], F32, tag="ps", name=f"S{b}_{i}_{jc}")
                for kp in range(NKP):
                    nc.tensor.matmul(
                        S_ps[:],
                        self.lhsT[:, 2 * kp:2 * kp + 2, i * P:(i + 1) * P],
                        self.rhsT[:, 2 * kp:2 * kp + 2, jc * CH:(jc + 1) * CH],
                        start=(kp == 0),
                        stop=(kp == NKP - 1),
                        perf_mode=DR,
                    )
                # += v[r] on DVE (keeps TensorE for real matmuls)
                nc.vector.tensor_add(
                    S_ps[:], S_ps[:], self.vb_sb[:, jc * CH:(jc + 1) * CH]
                )
                T_scr = pool_T.tile([P, CH], F32, tag="T", name=f"T{b}_{i}_{jc}")
                nc.scalar.activation(
                    T_scr[:],
                    S_ps[:],
                    AF.Tanh,
                    bias=self.u_sb[:, i:i + 1],
                    scale=1.0 / SCL,
                )
                self.pend_exp.append((jc, i, T_scr))
                if len(self.pend_exp) > lag:
                    self.flush_exp()

            # att_rhs[r, d] = (1/colsum[r]) * sum_l E[l,r] lhs[l,d]
            # 8-transpose bursts alternate with the matmul groups; colsum
            # rides the E^T copy's accum_out.
            def att_rhs_j(self, j):
                b = self.b
                E, E_T = self.E, self.E_T
                # fp8 PE transpose requires output element step 2: all 8
                # l-tiles of column j fill one bank ([P, SEQ, 2] fp8 = 2KB),
                # evacuated by a single strided full-row copy whose
                # accum_out is colsum[j] directly.
                pt = psum_tr.tile([P, SEQ, 2], F8, tag="ptr", name=f"pte{b}_{j}")
                for i in range(NT):
                    nc.tensor.transpose(
                        pt[:, i * P:(i + 1) * P, 0],
                        E[:, i, j * P:(j + 1) * P],
                        ident[:],
                    )
                nc.scalar.activation(
                    E_T[:, j, :],
                    pt[:, :, 0],
                    AF.Copy,
                    accum_out=self.r_col[:, j:j + 1],
                )
                nc.vector.reciprocal(self.r_col[:, j:j + 1], self.r_col[:, j:j + 1])
                po_tiles = []
                for dc in range(NCH):
                    po = psum_o.tile([P, CH], F32, tag="po", name=f"por{b}_{j}_{dc}")
                    for kp in range(NKP):
                        nc.tensor.matmul(
                            po[:],
                            E[:, 2 * kp:2 * kp + 2, j * P:(j + 1) * P],
                            self.lhs_nat[:, 2 * kp:2 * kp + 2, dc * CH:(dc + 1) * CH],
                            start=(kp == 0),
                            stop=(kp == NKP - 1),
                            perf_mode=DR,
                        )
                    po_tiles.append(po)
                osb = pool_out.tile([P, SEQ], BF16, tag="osb", name=f"or{b}_{j}")
                for dc in range(NCH):
                    nc.vector.tensor_scalar_mul(
                        osb[:, dc * CH:(dc + 1) * CH], po_tiles[dc][:],
                        self.r_col[:, j:j + 1],
                    )
                nc.sync.dma_start(att_rhs[b, j * P:(j + 1) * P, :], osb[:])

            def rowsum(self):
                b = self.b
                rowsum = pool_sm.tile([P, NT], F32, tag="rowsum", name=f"rowsum{b}")
                nc.vector.tensor_add(rowsum[:], self.rparts[:, 0, :], self.rparts[:, 1, :])
                self.r_row = pool_sm.tile([P, NT], F32, tag="rrow", name=f"rrow{b}")
                nc.vector.reciprocal(self.r_row[:], rowsum[:])

            # att_lhs[l, d] = (1/rowsum[l]) * sum_r E[l,r] rhs[r,d]
            # (output DMA per d-half so the dc=0 half ships while dc=1
            # computes -- shortens the kernel tail)
            def att_lhs_i(self, i):
                b = self.b
                osb = pool_out.tile([P, SEQ], BF16, tag="osb", name=f"ol{b}_{i}")
                for dc in range(NCH):
                    po = psum_o.tile([P, CH], F32, tag="po", name=f"pol{b}_{i}_{dc}")
                    for kp in range(NKP):
                        nc.tensor.matmul(
                            po[:],
                            self.E_T[:, 2 * kp:2 * kp + 2, i * P:(i + 1) * P],
                            self.rhs_nat[:, 2 * kp:2 * kp + 2, dc * CH:(dc + 1) * CH],
                            start=(kp == 0),
                            stop=(kp == NKP - 1),
                            perf_mode=DR,
                        )
                    # evacuation split ACT/DVE so the two d-halves scale out
                    # concurrently and the po banks recycle faster
                    if dc == 0:
                        nc.scalar.mul(
                            osb[:, dc * CH:(dc + 1) * CH], po[:], self.r_row[:, i:i + 1]
                        )
                    else:
                        nc.vector.tensor_scalar_mul(
                            osb[:, dc * CH:(dc + 1) * CH], po[:], self.r_row[:, i:i + 1]
                        )
                    nc.sync.dma_start(
                        att_lhs[b, i * P:(i + 1) * P, dc * CH:(dc + 1) * CH],
                        osb[:, dc * CH:(dc + 1) * CH],
                    )

            def score_and_att_rhs(self):
                """score (jc=0 then jc=1, with the r-half-0 att_rhs columns
                interleaved into the jc=1 pass so the PE never drains ahead
                of the ACT tanh/exp chain), then the remaining att_rhs."""
                for i in range(NT):
                    self.score_chunk(0, i)
                for i in range(NT):
                    self.score_chunk(1, i)
                    if i == 5:
                        self.att_rhs_j(0)
                    elif i == 7:
                        self.att_rhs_j(1)
                while self.pend_exp:
                    self.flush_exp()
                for j in range(2, 4):
                    self.att_rhs_j(j)
                self.rowsum()
                for j in range(4, NT):
                    self.att_rhs_j(j)

        b0 = Batch(0, first=True)
        b1 = Batch(1, first=False)
        b0.score_and_att_rhs()
        # bridge the att_rhs -> att_lhs dependency stall (att_lhs needs the
        # last E^T copy) with the first two score chunks of the next batch
        b1.score_chunk(0, 0)
        b1.score_chunk(0, 1)
        b1.score_chunk(0, 2)
        for i in range(NT):
            b0.att_lhs_i(i)
        for i in range(3, NT):
            b1.score_chunk(0, i)
        for i in range(NT):
            b1.score_chunk(1, i)
            if i == 5:
                b1.att_rhs_j(0)
            elif i == 7:
                b1.att_rhs_j(1)
        while b1.pend_exp:
            b1.flush_exp()
        for j in range(2, 4):
            b1.att_rhs_j(j)
        b1.rowsum()
        for j in range(4, NT):
            b1.att_rhs_j(j)
        for i in range(NT):
            b1.att_lhs_i(i)

        # warmup sink: a DRAM write keeps the warmup chain live; emitted
        # last so no real DMA ever queues behind the warmup dependency
        warm_dram = pool_dram.tile([P, P], F8, tag="warm", name="warm_dram")
        nc.sync.dma_start(warm_dram[:], wsb[:])

    nc.compile()
    return nc


def _get_nc():
    global _nc_cache
    if _nc_cache is None:
        _nc_cache = _build_program()
    return _nc_cache


def _prepare_in_maps(lhs, rhs, w, b):
    lhs = np.ascontiguousarray(lhs, dtype=np.float32)
    rhs = np.ascontiguousarray(rhs, dtype=np.float32)
    w = np.asarray(w, dtype=np.float32)
    b = np.float32(b)
    w_prod, w_l, w_r = w[:D], w[D:2 * D], w[2 * D:]

    # tiny host matvecs (exact, fp32)
    u_full = lhs @ w_l + b  # (N, L)
    v_full = rhs @ w_r      # (N, R)

    f8 = ml_dtypes.float8_e4m3
    bf = ml_dtypes.bfloat16
    id8 = np.eye(P, dtype=f8)
    lhs_n8 = lhs.astype(f8)
    rhs_n8 = rhs.astype(f8)
    # d-major score operands; w_prod sqrt-split over both sides, 8x
    # per-side scale keeps e4m3 operands in the normal range.
    sq = np.sqrt(np.abs(w_prod))
    lhs_t8 = np.ascontiguousarray(
        (lhs * (np.sign(w_prod) * sq * 8.0)).transpose(0, 2, 1)
    ).astype(f8)
    rhs_t8 = np.ascontiguousarray((rhs * (sq * 8.0)).transpose(0, 2, 1)).astype(f8)

    in_maps = []
    for c in range(N_CORES):
        b0 = c * NB
        u_arr = np.ascontiguousarray(
            u_full[b0:b0 + NB].reshape(NB, NT, P).transpose(0, 2, 1)
        )  # (NB, 128, 8)
        v_bf = (v_full[b0:b0 + NB] * SCL).astype(bf)  # (NB, R)
        vb_arr = np.ascontiguousarray(
            np.broadcast_to(v_bf[:, None, :], (NB, P, SEQ))
        )
        in_maps.append(
            {
                "lhs_n": lhs_n8[b0:b0 + NB],
                "rhs_n": rhs_n8[b0:b0 + NB],
                "lhs_t": lhs_t8[b0:b0 + NB],
                "rhs_t": rhs_t8[b0:b0 + NB],
                "u": u_arr,
                "vb": vb_arr,
                "id8": id8,
            }
        )
    return in_maps


def run_device(lhs, rhs, w, b, trace=False):
    """Returns (att_lhs, att_rhs, BassKernelResults)."""
    nc = _get_nc()
    in_maps = _prepare_in_maps(lhs, rhs, w, b)
    res = run_bass_kernel_spmd(
        nc, in_maps, core_ids=list(range(N_CORES)), trace=trace
    )
    N = lhs.shape[0]
    att_lhs = np.empty((N, SEQ, D), dtype=np.float32)
    att_rhs = np.empty((N, SEQ, D), dtype=np.float32)
    for c in range(N_CORES):
        b0 = c * NB
        att_lhs[b0:b0 + NB] = res.results[c]["att_lhs"].astype(np.float32)
        att_rhs[b0:b0 + NB] = res.results[c]["att_rhs"].astype(np.float32)
    return att_lhs, att_rhs, res


def kernel(lhs, rhs, w, b):
    import os

    lhs = np.asarray(lhs, dtype=np.float32)
    rhs = np.asarray(rhs, dtype=np.float32)
    assert lhs.shape == (N_CORES * NB, SEQ, D) and rhs.shape == lhs.shape, (
        f"expected ({N_CORES * NB}, {SEQ}, {D}) inputs, got {lhs.shape}/{rhs.shape}"
    )
    had = os.environ.get("BASS_NEVER_TRACE")
    os.environ["BASS_NEVER_TRACE"] = "1"
    try:
        att_lhs, att_rhs, _ = run_device(lhs, rhs, w, b, trace=False)
    finally:
        if had is None:
            os.environ.pop("BASS_NEVER_TRACE", None)
        else:
            os.environ["BASS_NEVER_TRACE"] = had
    lhs_out = np.concatenate([lhs, att_lhs], axis=2)
    rhs_out = np.concatenate([rhs, att_rhs], axis=2)
    return lhs_out, rhs_out
